# revision 38
# baseline (speedup 1.0000x reference)
"""GCN (EAConv) 2-layer kernel for Trainium2, 8 NeuronCores — v2.

Math: z = A @ relu((A @ x) @ W1 + b1) @ W2 + b2, A = D^-1/2 (Adj+I) D^-1/2.
Factorized normalization: w_e = dinv[src]*dinv[dst] is never materialized per
edge. dinv[src] is folded into the gather tables (x pre-scaled during the
bf16 precast; t scaled when written), dinv[dst] is applied per window on the
output side (column scale for layer 1, per-partition ACT scale for layer 2,
with the bias pre-multiplied by sqrt(deg) so it survives the scale).
Selection matrices are therefore pure one-hots: ONE DVE is_equal per
(group, half) instead of two tensor_tensor passes.

Sharding: destination nodes -> 8 cores x wpc windows of 128 slots, greedy-
balanced on per-window in-edge counts split by source half. Per-core windows
are sorted by in-edge count so the shared (SPMD) per-window chunk counts
kv[w] = max over cores track each core's true need (variable chunks, ~10%
fewer than a global K cap). Weights replicated. The intermediate t = z1@W2
is exchanged with NSLAB chunked AllGathers issued as window slabs complete,
overlapping the collective under phase-1 compute.

Aggregation: one batched gpsimd.dma_gather per (group, half) pulls source
rows (edge-slot order) into G; DVE builds one-hot Sel via broadcast
is_equal; PE accumulates Sel/G chunk matmuls into PSUM per window. Dense
transforms run in bf16 on the PE.

Host-side preprocessing touches ONLY edge_index (graph structure): degrees,
node->slot permutation, edge->slot packing, int16 index tables, dinv/sqdeg
vectors. All math on x_all/W1/b1/W2/b2 runs on device.
"""
import os
import sys
import math

for _p in ("/opt/trn_rl_repo", "/root/.axon_site/_ro/trn_rl_repo"):
    if os.path.isdir(_p) and _p not in sys.path:
        sys.path.insert(0, _p)

import numpy as np
import ml_dtypes

import concourse.bass as bass
import concourse.bacc as bacc
import concourse.tile as tile
from concourse import mybir
from concourse.bass_utils import run_bass_kernel_spmd

P = 128
N_CORES = 8
GROUP_W = 3          # windows per gather group
NSLAB = 4            # chunked AllGather slabs

TRACE = False
LAST = {}            # stats from last run (exec_time_ns etc.)
_CACHE = {}


# ---------------------------------------------------------------- preprocess
def _preprocess(edge_index, n_nodes, n_cores=N_CORES):
    src = np.asarray(edge_index[0]).astype(np.int64)
    dst = np.asarray(edge_index[1]).astype(np.int64)
    N = n_nodes
    half_x = N // 2
    deg = np.bincount(dst, minlength=N).astype(np.float64) + 1.0
    dinv = (1.0 / np.sqrt(deg)).astype(np.float32)
    sqdeg = np.sqrt(deg).astype(np.float32)
    loop = np.arange(N, dtype=np.int64)
    asrc = np.concatenate([src, loop])
    adst = np.concatenate([dst, loop])
    eh1 = (asrc >= half_x).astype(np.int64)       # layer-1 table half

    w0 = np.bincount(adst[eh1 == 0], minlength=N)
    w1 = np.bincount(adst[eh1 == 1], minlength=N)

    wpc = int(math.ceil(N / n_cores / P))
    spc = wpc * P
    nwin_half = (n_cores // 2) * wpc
    assert nwin_half * P >= half_x and nwin_half * P >= (N - half_x)

    # greedy balance nodes into global windows (within x-half so layer-1
    # source halves stay balanced per window)
    win_of = np.empty(N, np.int64)
    pos_of = np.empty(N, np.int64)
    for h in (0, 1):
        nodes = np.nonzero((np.arange(N) >= half_x) == bool(h))[0]
        order = nodes[np.argsort(-(w0[nodes] + w1[nodes]), kind="stable")]
        s0 = np.zeros(nwin_half)
        s1 = np.zeros(nwin_half)
        cnt = np.zeros(nwin_half, np.int64)
        for n in order:
            score = np.maximum(s0 + w0[n], s1 + w1[n])
            score[cnt >= P] = np.inf
            b = int(np.argmin(score))
            win_of[n] = h * nwin_half + b
            pos_of[n] = cnt[b]
            cnt[b] += 1
            s0[b] += w0[n]
            s1[b] += w1[n]

    core_of = win_of // wpc
    lw_of = win_of % wpc                     # pre-perm local window

    # per-core window permutation: sort local windows by in-edge count desc
    tot = np.zeros((n_cores, wpc), np.int64)
    np.add.at(tot, (core_of[adst], lw_of[adst]), 1)
    nw_map = np.empty((n_cores, wpc), np.int64)
    for c in range(n_cores):
        order = np.argsort(-tot[c], kind="stable")
        nw_map[c, order] = np.arange(wpc)
    nw_of = nw_map[core_of, lw_of]           # sorted local window index
    slot_in_core = nw_of * P + pos_of

    # inverse map (core, nw, pos) -> node (sentinel N for empty slots)
    nodes_of = np.full((n_cores, spc), N, np.int64)
    nodes_of[core_of, slot_in_core] = np.arange(N)
    dinv_ext = np.r_[dinv, np.float32(0.0)]
    sqdeg_ext = np.r_[sqdeg, np.float32(0.0)]

    # slab partition of windows for the chunked AllGather (small last slab so
    # the final, phase-2-gating collective is short)
    last = max(1, wpc // 24)
    rest = wpc - last
    base, rem = rest // (NSLAB - 1), rest % (NSLAB - 1)
    ws = [base + (1 if i < rem else 0) for i in range(NSLAB - 1)] + [last]
    sstart = np.r_[0, np.cumsum(ws)[:-1]].astype(np.int64)
    outbase = np.r_[0, np.cumsum([n_cores * w * P for w in ws])[:-1]].astype(np.int64)
    slab_of_w = np.repeat(np.arange(NSLAB), ws)

    # slab-major cc row of each node (layer-2 table address)
    s_n = slab_of_w[nw_of]
    ccrow = (outbase[s_n] + core_of * (np.array(ws)[s_n] * P)
             + (nw_of - sstart[s_n]) * P + pos_of)
    tot_rows = n_cores * spc
    half_cc = tot_rows // 2
    assert half_cc <= 32767 and half_x <= 32767
    eh2 = (ccrow[asrc] >= half_cc).astype(np.int64)

    # per (core, nw, half) counts for both layers
    ecore = core_of[adst]
    enw = nw_of[adst]
    def counts(eh):
        c = np.zeros((n_cores, wpc, 2), np.int64)
        np.add.at(c, (ecore, enw, eh), 1)
        return c
    c1 = counts(eh1)
    c2 = counts(eh2)
    kv = {}
    for L, c in ((1, c1), (2, c2)):
        for h in (0, 1):
            kv[(L, h)] = np.ceil(c[:, :, h].max(axis=0) / P).astype(np.int64)

    groups = []
    a = 0
    while a < wpc:
        b = min(a + GROUP_W, wpc)
        groups.append((a, b))
        a = b

    # edge -> slot tables, packed per (layer, half): per group g the columns
    # are [idx wrapped (8*nch) | ds as bf16-bits (nch)] int16
    # Layer-1 table rows are permuted to row = (n%128)*ntile + n//128 so the
    # precast can write xg with one contiguous descriptor per partition.
    half_rows = ((half_x + P - 1) // P) * P
    ntile = half_rows // P
    relx = asrc - eh1 * half_x
    relidx = {1: (relx % P) * ntile + relx // P,
              2: (ccrow[asrc] - eh2 * half_cc)}
    ehs = {1: eh1, 2: eh2}
    packed = {}
    nchg = {}
    for L in (1, 2):
        for h in (0, 1):
            k = kv[(L, h)]
            cb = np.r_[0, np.cumsum(k)]
            totch = int(cb[-1])
            idxf = np.zeros((n_cores, totch * P), np.int16)
            dsf = np.full((n_cores, totch * P), -1.0, np.float32)
            m = ehs[L] == h
            c = ecore[m]
            w = enw[m]
            key = c * wpc + w
            # sort by source within each (core, window) bucket: consecutive
            # gather descriptors then walk the table monotonically, which is
            # much friendlier to HBM banks than random order
            order = np.lexsort((relidx[L][m], key))
            kcnt = np.bincount(key, minlength=n_cores * wpc)
            kst = np.r_[0, np.cumsum(kcnt)[:-1]]
            rank = np.empty(len(key), np.int64)
            rank[order] = np.arange(len(key)) - kst[key[order]]
            flat = (cb[w] + rank // P) * P + rank % P
            idxf[c, flat] = relidx[L][m].astype(np.int16)
            dsf[c, flat] = pos_of[adst[m]].astype(np.float32)

            ng = [int(k[a:b].sum()) for (a, b) in groups]
            nchg[(L, h)] = ng
            cols = sum(9 * n for n in ng)
            pk = np.zeros((n_cores, P, cols), np.int16)
            dsb = dsf.astype(ml_dtypes.bfloat16).view(np.int16)
            co = 0
            for gi, (a, b) in enumerate(groups):
                n = ng[gi]
                if n == 0:
                    continue
                s0, s1 = int(cb[a]) * P, int(cb[b]) * P
                w16 = idxf[:, s0:s1].reshape(n_cores, -1, 16).transpose(0, 2, 1)
                pk[:, :, co:co + 8 * n] = np.tile(w16, (1, 8, 1))
                pk[:, :, co + 8 * n:co + 9 * n] = \
                    dsf[:, s0:s1].astype(ml_dtypes.bfloat16).view(np.int16) \
                       .reshape(n_cores, n, P).transpose(0, 2, 1)
                co += 9 * n
            packed[(L, h)] = np.ascontiguousarray(pk)

    # per-core window constants
    nod = nodes_of.reshape(n_cores, wpc, P)
    Dw = np.broadcast_to(dinv_ext[nod].reshape(n_cores, 1, spc),
                         (n_cores, P, spc)).astype(np.float32)
    dinvw = dinv_ext[nod].transpose(0, 2, 1).astype(np.float32)  # [c,128,wpc]
    sqdegr = sqdeg_ext[nod].reshape(n_cores, 1, spc).astype(np.float32)

    # precast dinv columns per half: [128, ntiles]
    dpre = np.zeros((2, P, ntile), np.float32)
    for h in (0, 1):
        base_r = h * half_x
        nrows = (N - half_x) if h else half_x
        idx = base_r + np.arange(ntile * P) % (ntile * P)
        v = np.zeros(ntile * P, np.float32)
        v[:nrows] = dinv[base_r:base_r + nrows]
        dpre[h] = v.reshape(ntile, P).T

    nchmax = max(max(nchg[(L, h)]) for L in (1, 2) for h in (0, 1))
    pre = {
        "NCHMAX": nchmax,
        "N": N, "half_x": half_x, "n_cores": n_cores, "wpc": wpc, "spc": spc,
        "half_cc": half_cc, "ws": ws, "sstart": sstart, "outbase": outbase,
        "groups": groups, "kv": kv, "nchg": nchg, "packed": packed,
        "Dw": Dw, "dinvw": dinvw, "sqdegr": sqdegr, "dpre": dpre,
        "core_of": core_of, "slot_in_core": slot_in_core,
        "tot_rows": tot_rows,
    }
    return pre


# ------------------------------------------------------------------- builder
def _build(pre, D, H):
    N = pre["N"]
    half_x = pre["half_x"]
    n_cores = pre["n_cores"]
    wpc = pre["wpc"]
    spc = pre["spc"]
    half_cc = pre["half_cc"]
    ws = pre["ws"]
    sstart = list(pre["sstart"])
    outbase = list(pre["outbase"])
    groups = pre["groups"]
    kv = pre["kv"]
    nchg = pre["nchg"]
    JH = H // P
    f32 = mybir.dt.float32
    gdt = mybir.dt.bfloat16
    i16 = mybir.dt.int16
    half_rows = ((half_x + P - 1) // P) * P
    ntile = half_rows // P

    NCHMAX = pre["NCHMAX"]

    nc = bacc.Bacc("TRN2", target_bir_lowering=False, debug=False,
                   num_devices=n_cores, num_swdge_queues=4)
    qctr = [0]

    def next_q():
        q = qctr[0] % 4
        qctr[0] += 1
        return q

    x = nc.dram_tensor("x_all", [N, D], f32, kind="ExternalInput").ap()
    W1 = nc.dram_tensor("W1", [D, H], f32, kind="ExternalInput").ap()
    b1c = nc.dram_tensor("b1c", [P, JH], f32, kind="ExternalInput").ap()
    W2 = nc.dram_tensor("W2", [H, D], f32, kind="ExternalInput").ap()
    b2r = nc.dram_tensor("b2r", [1, D], f32, kind="ExternalInput").ap()
    iota_in = nc.dram_tensor("iota_in", [P, P], gdt, kind="ExternalInput").ap()
    pk = {}
    for L in (1, 2):
        for h in (0, 1):
            cols = pre["packed"][(L, h)].shape[2]
            pk[(L, h)] = nc.dram_tensor(f"pk{L}{h}", [P, max(cols, 1)], i16,
                                        kind="ExternalInput").ap()
    Dw_d = nc.dram_tensor("Dw", [P, spc], f32, kind="ExternalInput").ap()
    dinvw_d = nc.dram_tensor("dinvw", [P, wpc], f32, kind="ExternalInput").ap()
    dpre0_d = nc.dram_tensor("dpre0", [P, ntile], f32, kind="ExternalInput").ap()
    dpre1_d = nc.dram_tensor("dpre1", [P, ntile], f32, kind="ExternalInput").ap()
    z_out = nc.dram_tensor("z_out", [spc, D], f32, kind="ExternalOutput").ap()

    with tile.TileContext(nc) as tc:
        with tc.tile_pool(name="const", bufs=1) as cst, \
             tc.tile_pool(name="pc", bufs=3) as pc, \
             tc.tile_pool(name="tb", bufs=5) as tbp, \
             tc.tile_pool(name="gg", bufs=4) as gg, \
             tc.tile_pool(name="sel", bufs=4) as sel, \
             tc.tile_pool(name="wk", bufs=3) as wk, \
             tc.tile_pool(name="psA", bufs=2, space="PSUM") as psA, \
             tc.tile_pool(name="psB", bufs=3, space="PSUM") as psB, \
             tc.tile_pool(name="dram", bufs=1, space="DRAM") as dram:

            # ---------------- constants (f32 staging via the recycled pc pool)
            SGK = 12
            W1f = pc.tile([P, SGK * P], f32, tag="xt", name="w1f")
            nc.sync.dma_start(out=W1f[:, :H], in_=W1)
            W1b = cst.tile([P, H], gdt)
            nc.vector.tensor_copy(out=W1b[:], in_=W1f[:, :H])
            W2f = pc.tile([P, SGK * P], f32, tag="xt", name="w2f")
            for j in range(JH):
                nc.sync.dma_start(out=W2f[:, j * D:(j + 1) * D],
                                  in_=W2[j * P:(j + 1) * P, :])
            W2b = cst.tile([P, JH * D], gdt)
            nc.vector.tensor_copy(out=W2b[:], in_=W2f[:, :JH * D])
            b1_sb = cst.tile([P, JH], f32)
            nc.sync.dma_start(out=b1_sb[:], in_=b1c)
            b2f = cst.tile([1, D], f32)
            nc.sync.dma_start(out=b2f[:], in_=b2r)
            b2b = cst.tile([1, D], gdt)
            nc.vector.tensor_copy(out=b2b[:], in_=b2f[:])
            iota_sb = cst.tile([P, P], gdt)
            nc.sync.dma_start(out=iota_sb[:], in_=iota_in)
            Dw_f = pc.tile([P, SGK * P], f32, tag="xt", name="dwf")
            Dw_sb = cst.tile([P, spc], gdt)
            for j in range(0, wpc, SGK):
                k = min(SGK, wpc - j)
                nc.sync.dma_start(out=Dw_f[:, :k * P],
                                  in_=Dw_d[:, j * P:(j + k) * P])
                nc.vector.tensor_copy(out=Dw_sb[:, j * P:(j + k) * P],
                                      in_=Dw_f[:, :k * P])
            dinvw_sb = cst.tile([P, wpc], f32)
            nc.sync.dma_start(out=dinvw_sb[:], in_=dinvw_d)
            ones1 = cst.tile([1, P], gdt)
            nc.vector.memset(ones1[:], 1.0)
            b2rep = cst.tile([P, P], f32)
            ps_b2 = psB.tile([P, P], f32, tag="pu", name="psb2")
            nc.tensor.matmul(ps_b2[:], lhsT=ones1[:], rhs=b2b[:],
                             start=True, stop=True)
            nc.vector.tensor_copy(out=b2rep[:], in_=ps_b2[:])
            dpre_sb = [cst.tile([P, ntile], f32, tag=f"dp{h}", name=f"dpre{h}")
                       for h in (0, 1)]
            nc.sync.dma_start(out=dpre_sb[0][:], in_=dpre0_d)
            nc.sync.dma_start(out=dpre_sb[1][:], in_=dpre1_d)

            xg = [dram.tile([half_rows, D], gdt, tag=f"xg{h}", name=f"xg{h}")
                  for h in (0, 1)]
            ccin = [dram.tile([ws[s] * P, D], gdt, tag=f"cci{s}", name=f"cci{s}")
                    for s in range(NSLAB)]
            ccout = [dram.tile([n_cores * ws[s] * P, D], gdt, tag=f"cco{s}",
                               name=f"cco{s}", addr_space="Shared")
                     for s in range(NSLAB)]
            cc = dram.tile([pre["tot_rows"], D], gdt)

            # ---------------- precast: xg[h] = bf16(dinv * x), per half
            sgi = [0]
            for h in (0, 1):
                base_r = h * half_x
                nrows = (N - half_x) if h else half_x
                sgs = []
                r = 0
                while r + SGK * P <= nrows:
                    sgs.append((r, SGK, P))
                    r += SGK * P
                if nrows - r >= P:
                    k = (nrows - r) // P
                    sgs.append((r, k, P))
                    r += k * P
                if nrows - r > 0:
                    sgs.append((r, 1, nrows - r))
                xgv = xg[h][:].rearrange("(p t) f -> p t f", t=ntile)
                for (r0, k, prow) in sgs:
                    xt = pc.tile([P, SGK * P], f32, tag="xt")
                    xb = pc.tile([P, SGK * P], gdt, tag="xb")
                    src_ap = x[base_r + r0: base_r + r0 + (k - 1) * P + prow, :]
                    nc.scalar.dma_start(
                        out=xt[:prow, :k * P].rearrange("p (k f) -> p k f", f=P),
                        in_=src_ap.rearrange("(k p) f -> p k f", p=P) if prow == P
                        else src_ap.rearrange("(k p) f -> p k f", p=prow))
                    t0 = r0 // P
                    dv = dpre_sb[h][:prow, t0:t0 + k].unsqueeze(2) \
                        .broadcast_to([prow, k, P])
                    eng = nc.vector if (sgi[0] % 2 == 0) else nc.gpsimd
                    sgi[0] += 1
                    eng.tensor_tensor(
                        out=xb[:prow, :k * P].rearrange("p (k f) -> p k f", f=P),
                        in0=xt[:prow, :k * P].rearrange("p (k f) -> p k f", f=P),
                        in1=dv, op=mybir.AluOpType.mult)
                    nc.sync.dma_start(
                        out=xgv[:prow, t0:t0 + k, :],
                        in_=xb[:prow, :k * P].rearrange("p (k f) -> p k f", f=P))

            # ---------------- aggregation sweep
            def agg_phase(L, in_aps, emit, group_end=None):
                cb = {h: np.r_[0, np.cumsum(kv[(L, h)])] for h in (0, 1)}
                coff = {h: 0 for h in (0, 1)}
                for gi, (a, b) in enumerate(groups):
                    Gs, Sels = {}, {}
                    for h in (0, 1):
                        n = nchg[(L, h)][gi]
                        if n == 0:
                            continue
                        tbt = tbp.tile([P, 9 * NCHMAX], i16, tag=f"tb{h}")
                        nc.sync.dma_start(
                            out=tbt[:, :9 * n],
                            in_=pk[(L, h)][:, coff[h]:coff[h] + 9 * n])
                        coff[h] += 9 * n
                        G = gg.tile([P, NCHMAX * P], gdt, tag=f"g{h}")
                        ca = max(1, n // 2)
                        for (c_lo, c_hi) in ((0, ca), (ca, n)):
                            if c_hi <= c_lo:
                                continue
                            nn = (c_hi - c_lo) * P
                            nc.gpsimd.dma_gather(
                                out_ap=G[:, c_lo * P:c_hi * P]
                                    .rearrange("p (k d) -> p k d", d=P),
                                in_ap=in_aps[h],
                                idxs_ap=tbt[:, c_lo * 8:c_hi * 8],
                                num_idxs=nn, num_idxs_reg=nn, elem_size=P,
                                single_packet=False, queue_num=next_q())
                        S = sel.tile([P, NCHMAX * P], gdt, tag=f"s{h}")
                        s3 = S[:, :n * P].rearrange("p (c j) -> p c j", j=P)
                        ds_ap = tbt[:, 8 * n:9 * n].bitcast(gdt)
                        d_b = ds_ap.unsqueeze(2).broadcast_to([P, n, P])
                        i_b = iota_sb[:, :P].unsqueeze(1).broadcast_to([P, n, P])
                        nc.vector.tensor_tensor(out=s3, in0=d_b, in1=i_b,
                                                op=mybir.AluOpType.is_equal)
                        Gs[h], Sels[h] = G, S
                    for w in range(a, b):
                        psum = psB.tile([P, P], f32, tag="pu")
                        ci = 0
                        for h in (0, 1):
                            if nchg[(L, h)][gi] == 0:
                                continue
                            k0 = int(cb[h][w] - cb[h][a])
                            for k in range(int(kv[(L, h)][w])):
                                off = (k0 + k) * P
                                gsl = Gs[h][:, off:off + P]
                                ssl = Sels[h][:, off:off + P]
                                if L == 1:
                                    nc.tensor.matmul(psum[:], lhsT=gsl, rhs=ssl,
                                                     start=(ci == 0), stop=False)
                                else:
                                    nc.tensor.matmul(psum[:], lhsT=ssl, rhs=gsl,
                                                     start=(ci == 0), stop=False)
                                ci += 1
                        emit(w, psum)
                    if group_end is not None:
                        group_end(gi, a, b)

            # ---- phase 1
            def emit_l1(w, psum_u):
                u_sb = wk.tile([P, P], gdt, tag="u")
                nc.vector.tensor_tensor(out=u_sb[:], in0=psum_u[:],
                                        in1=Dw_sb[:, w * P:(w + 1) * P],
                                        op=mybir.AluOpType.mult)
                psz1 = psA.tile([P, H], f32, tag="pz1")
                z1 = wk.tile([P, H], gdt, tag="z1")
                for j in range(JH):
                    nc.tensor.matmul(psz1[:, j * P:(j + 1) * P],
                                     lhsT=W1b[:, j * P:(j + 1) * P],
                                     rhs=u_sb[:], start=True, stop=True)
                    nc.scalar.activation(out=z1[:, j * P:(j + 1) * P],
                                         in_=psz1[:, j * P:(j + 1) * P],
                                         func=mybir.ActivationFunctionType.Relu,
                                         bias=b1_sb[:, j:j + 1])
                pst = psB.tile([P, P], f32, tag="pt")
                for j in range(JH):
                    nc.tensor.matmul(pst[:],
                                     lhsT=z1[:, j * P:(j + 1) * P],
                                     rhs=W2b[:, j * D:(j + 1) * D],
                                     start=(j == 0), stop=(j == JH - 1))
                t_sb = wk.tile([P, D], gdt, tag="t")
                nc.scalar.activation(out=t_sb[:], in_=pst[:],
                                     func=mybir.ActivationFunctionType.Copy,
                                     scale=dinvw_sb[:, w:w + 1])
                s = int(np.searchsorted(np.r_[sstart[1:], wpc], w, side="right"))
                wl = w - sstart[s]
                nc.scalar.dma_start(out=ccin[s][wl * P:(wl + 1) * P, :],
                                    in_=t_sb[:])
                if w == sstart[s] + ws[s] - 1:
                    nc.gpsimd.collective_compute(
                        "AllGather", mybir.AluOpType.bypass,
                        replica_groups=[list(range(n_cores))],
                        ins=[ccin[s][:]], outs=[ccout[s][:]])
                    nc.scalar.dma_start(
                        out=cc[outbase[s]:outbase[s] + n_cores * ws[s] * P, :],
                        in_=ccout[s][:])

            agg_phase(1, [xg[0][:], xg[1][:]], emit_l1)

            # ---- phase 2
            zg_box = {}

            def emit_l2(w, psum_z):
                gi = w // GROUP_W
                a = groups[gi][0]
                if w == a:
                    zg_box["t"] = wk.tile([P, GROUP_W * P], f32, tag="zg",
                                          name="zg")
                nc.vector.scalar_tensor_tensor(
                    out=zg_box["t"][:, (w - a) * P:(w - a + 1) * P],
                    in0=psum_z[:], scalar=dinvw_sb[:, w:w + 1], in1=b2rep[:],
                    op0=mybir.AluOpType.mult, op1=mybir.AluOpType.add)

            def group_end_l2(gi, a, b):
                zg = zg_box["t"]
                nc.scalar.dma_start(
                    out=z_out[a * P:b * P, :].rearrange("(w p) f -> p w f", p=P),
                    in_=zg[:, :(b - a) * P].rearrange("p (w f) -> p w f", f=P))

            agg_phase(2, [cc[0:half_cc, :], cc[half_cc:, :]], emit_l2,
                      group_end=group_end_l2)

    nc.compile()
    return nc


# -------------------------------------------------------------------- kernel
def kernel(x_all, W1, b1, W2, b2, edge_index, ix=0, max_iter=10):
    x_all = np.ascontiguousarray(np.asarray(x_all, dtype=np.float32))
    W1 = np.ascontiguousarray(np.asarray(W1, dtype=np.float32))
    b1 = np.ascontiguousarray(np.asarray(b1, dtype=np.float32))
    W2 = np.ascontiguousarray(np.asarray(W2, dtype=np.float32))
    b2 = np.ascontiguousarray(np.asarray(b2, dtype=np.float32))
    edge_index = np.asarray(edge_index)

    N, D = x_all.shape
    H = W1.shape[1]
    ekey = (N, D, H, edge_index.shape[1], GROUP_W,
            int(edge_index[0, 0]), int(edge_index[1, -1]))
    if ekey in _CACHE:
        nc, pre = _CACHE[ekey]
    else:
        pre = _preprocess(edge_index, N)
        nc = _build(pre, D, H)
        _CACHE[ekey] = (nc, pre)

    JH = H // P
    b1c = b1.reshape(JH, P).T.copy()
    b2r = b2.reshape(1, D).copy()
    iota = np.ascontiguousarray(
        np.broadcast_to(np.arange(P, dtype=np.float32)[None, None, :],
                        (P, pre["NCHMAX"], P)).reshape(P, -1)) \
        .astype(ml_dtypes.bfloat16)

    in_maps = []
    for c in range(pre["n_cores"]):
        im = {"x_all": x_all, "W1": W1, "b1c": b1c, "W2": W2, "b2r": b2r,
              "iota_in": iota,
              "Dw": pre["Dw"][c], "dinvw": pre["dinvw"][c],
              "dpre0": pre["dpre"][0], "dpre1": pre["dpre"][1]}
        for L in (1, 2):
            for h in (0, 1):
                arr = pre["packed"][(L, h)][c]
                if arr.shape[1] == 0:
                    arr = np.zeros((P, 1), np.int16)
                im[f"pk{L}{h}"] = arr
        in_maps.append(im)

    res = run_bass_kernel_spmd(nc, in_maps, core_ids=list(range(pre["n_cores"])),
                               trace=TRACE)
    LAST["exec_time_ns"] = res.exec_time_ns
    LAST["mean_exec_time_ns"] = res.mean_exec_time_ns
    LAST["per_core_scope_times"] = res.per_core_scope_times
    LAST["trace_path"] = (res.instructions_and_trace or (None, None))[1]
    LAST["profile_json"] = res.profile_json

    zs = np.stack([res.results[c]["z_out"] for c in range(pre["n_cores"])])
    z = zs[pre["core_of"], pre["slot_in_core"]]
    return z.astype(np.float32)


if __name__ == "__main__":
    rng = np.random.default_rng(0)
    N, E, D, H = 4096, 40000, 128, 512
    ei = rng.integers(0, N, size=(2, E)).astype(np.int64)
    x = rng.standard_normal((N, D), dtype=np.float32)
    W1 = rng.standard_normal((D, H), dtype=np.float32) / np.sqrt(D)
    b1 = rng.standard_normal(H).astype(np.float32) * 0.1
    W2 = rng.standard_normal((H, D), dtype=np.float32) / np.sqrt(H)
    b2 = rng.standard_normal(D).astype(np.float32) * 0.1

    deg = np.bincount(ei[1], minlength=N) + 1.0
    dinv = 1.0 / np.sqrt(deg)
    asrc = np.concatenate([ei[0], np.arange(N)])
    adst = np.concatenate([ei[1], np.arange(N)])
    nrm = dinv[asrc] * dinv[adst]

    def agg(t):
        out = np.zeros_like(t)
        np.add.at(out, adst, t[asrc] * nrm[:, None])
        return out

    z1 = np.maximum(agg(x.astype(np.float64)) @ W1 + b1, 0)
    ref = agg(z1 @ W2) + b2

    got = kernel(x, W1, b1, W2, b2, ei)
    err = np.abs(got - ref)
    rel = err.max() / np.abs(ref).max()
    print(f"exec_time_ns: {LAST['exec_time_ns']}")
    print(f"max abs err {err.max():.3e}  rel(absmax) {rel:.3e}")


# revision 39
# speedup vs baseline: 1.1175x; 1.1175x over previous
"""GCN (EAConv) 2-layer kernel for Trainium2, 8 NeuronCores — v2.

Math: z = A @ relu((A @ x) @ W1 + b1) @ W2 + b2, A = D^-1/2 (Adj+I) D^-1/2.
Factorized normalization: w_e = dinv[src]*dinv[dst] is never materialized per
edge. dinv[src] is folded into the gather tables (x pre-scaled during the
bf16 precast; t scaled when written), dinv[dst] is applied per window on the
output side (column scale for layer 1, per-partition ACT scale for layer 2,
with the bias pre-multiplied by sqrt(deg) so it survives the scale).
Selection matrices are therefore pure one-hots: ONE DVE is_equal per
(group, half) instead of two tensor_tensor passes.

Sharding: destination nodes -> 8 cores x wpc windows of 128 slots, greedy-
balanced on per-window in-edge counts split by source half. Per-core windows
are sorted by in-edge count so the shared (SPMD) per-window chunk counts
kv[w] = max over cores track each core's true need (variable chunks, ~10%
fewer than a global K cap). Weights replicated. The intermediate t = z1@W2
is exchanged with NSLAB chunked AllGathers issued as window slabs complete,
overlapping the collective under phase-1 compute.

Aggregation: one batched gpsimd.dma_gather per (group, half) pulls source
rows (edge-slot order) into G; DVE builds one-hot Sel via broadcast
is_equal; PE accumulates Sel/G chunk matmuls into PSUM per window. Dense
transforms run in bf16 on the PE.

Host-side preprocessing touches ONLY edge_index (graph structure): degrees,
node->slot permutation, edge->slot packing, int16 index tables, dinv/sqdeg
vectors. All math on x_all/W1/b1/W2/b2 runs on device.
"""
import os
import sys
import math

for _p in ("/opt/trn_rl_repo", "/root/.axon_site/_ro/trn_rl_repo"):
    if os.path.isdir(_p) and _p not in sys.path:
        sys.path.insert(0, _p)

import numpy as np
import ml_dtypes

import concourse.bass as bass
import concourse.bacc as bacc
import concourse.tile as tile
from concourse import mybir
from concourse.bass_utils import run_bass_kernel_spmd

P = 128
N_CORES = 8
GROUP_W = 3          # windows per gather group
NSLAB = 4            # chunked AllGather slabs

TRACE = False
LAST = {}            # stats from last run (exec_time_ns etc.)
_CACHE = {}


# ---------------------------------------------------------------- preprocess
def _preprocess(edge_index, n_nodes, n_cores=N_CORES):
    src = np.asarray(edge_index[0]).astype(np.int64)
    dst = np.asarray(edge_index[1]).astype(np.int64)
    N = n_nodes
    half_x = N // 2
    deg = np.bincount(dst, minlength=N).astype(np.float64) + 1.0
    dinv = (1.0 / np.sqrt(deg)).astype(np.float32)
    sqdeg = np.sqrt(deg).astype(np.float32)
    loop = np.arange(N, dtype=np.int64)
    asrc = np.concatenate([src, loop])
    adst = np.concatenate([dst, loop])
    eh1 = (asrc >= half_x).astype(np.int64)       # layer-1 table half

    w0 = np.bincount(adst[eh1 == 0], minlength=N)
    w1 = np.bincount(adst[eh1 == 1], minlength=N)

    wpc = int(math.ceil(N / n_cores / P))
    spc = wpc * P
    nwin_half = (n_cores // 2) * wpc
    assert nwin_half * P >= half_x and nwin_half * P >= (N - half_x)

    # greedy balance nodes into global windows (within x-half so layer-1
    # source halves stay balanced per window)
    win_of = np.empty(N, np.int64)
    pos_of = np.empty(N, np.int64)
    for h in (0, 1):
        nodes = np.nonzero((np.arange(N) >= half_x) == bool(h))[0]
        order = nodes[np.argsort(-(w0[nodes] + w1[nodes]), kind="stable")]
        s0 = np.zeros(nwin_half)
        s1 = np.zeros(nwin_half)
        cnt = np.zeros(nwin_half, np.int64)
        for n in order:
            score = np.maximum(s0 + w0[n], s1 + w1[n])
            score[cnt >= P] = np.inf
            b = int(np.argmin(score))
            win_of[n] = h * nwin_half + b
            pos_of[n] = cnt[b]
            cnt[b] += 1
            s0[b] += w0[n]
            s1[b] += w1[n]

    core_of = win_of // wpc
    lw_of = win_of % wpc                     # pre-perm local window

    # per-core window permutation: sort local windows by in-edge count desc
    tot = np.zeros((n_cores, wpc), np.int64)
    np.add.at(tot, (core_of[adst], lw_of[adst]), 1)
    nw_map = np.empty((n_cores, wpc), np.int64)
    for c in range(n_cores):
        order = np.argsort(-tot[c], kind="stable")
        nw_map[c, order] = np.arange(wpc)
    nw_of = nw_map[core_of, lw_of]           # sorted local window index
    slot_in_core = nw_of * P + pos_of

    # inverse map (core, nw, pos) -> node (sentinel N for empty slots)
    nodes_of = np.full((n_cores, spc), N, np.int64)
    nodes_of[core_of, slot_in_core] = np.arange(N)
    dinv_ext = np.r_[dinv, np.float32(0.0)]
    sqdeg_ext = np.r_[sqdeg, np.float32(0.0)]

    # slab partition of windows for the chunked AllGather (small last slab so
    # the final, phase-2-gating collective is short)
    last = max(1, wpc // 24)
    rest = wpc - last
    base, rem = rest // (NSLAB - 1), rest % (NSLAB - 1)
    ws = [base + (1 if i < rem else 0) for i in range(NSLAB - 1)] + [last]
    sstart = np.r_[0, np.cumsum(ws)[:-1]].astype(np.int64)
    outbase = np.r_[0, np.cumsum([n_cores * w * P for w in ws])[:-1]].astype(np.int64)
    slab_of_w = np.repeat(np.arange(NSLAB), ws)

    # slab-major cc row of each node (layer-2 table address)
    s_n = slab_of_w[nw_of]
    ccrow = (outbase[s_n] + core_of * (np.array(ws)[s_n] * P)
             + (nw_of - sstart[s_n]) * P + pos_of)
    tot_rows = n_cores * spc
    half_cc = tot_rows // 2
    assert half_cc <= 32767 and half_x <= 32767
    eh2 = (ccrow[asrc] >= half_cc).astype(np.int64)

    # per (core, nw, half) counts for both layers
    ecore = core_of[adst]
    enw = nw_of[adst]
    def counts(eh):
        c = np.zeros((n_cores, wpc, 2), np.int64)
        np.add.at(c, (ecore, enw, eh), 1)
        return c
    c1 = counts(eh1)
    c2 = counts(eh2)
    kv = {}
    for L, c in ((1, c1), (2, c2)):
        for h in (0, 1):
            kv[(L, h)] = np.ceil(c[:, :, h].max(axis=0) / P).astype(np.int64)

    groups = []
    a = 0
    while a < wpc:
        b = min(a + GROUP_W, wpc)
        groups.append((a, b))
        a = b

    # edge -> slot tables, packed per (layer, half): per group g the columns
    # are [idx wrapped (8*nch) | ds as bf16-bits (nch)] int16
    # Layer-1 table rows are permuted to row = (n%128)*ntile + n//128 so the
    # precast can write xg with one contiguous descriptor per partition.
    half_rows = ((half_x + P - 1) // P) * P
    ntile = half_rows // P
    relx = asrc - eh1 * half_x
    relidx = {1: (relx % P) * ntile + relx // P,
              2: (ccrow[asrc] - eh2 * half_cc)}
    ehs = {1: eh1, 2: eh2}
    packed = {}
    nchg = {}
    for L in (1, 2):
        for h in (0, 1):
            k = kv[(L, h)]
            cb = np.r_[0, np.cumsum(k)]
            totch = int(cb[-1])
            idxf = np.zeros((n_cores, totch * P), np.int16)
            dsf = np.full((n_cores, totch * P), -1.0, np.float32)
            m = ehs[L] == h
            c = ecore[m]
            w = enw[m]
            key = c * wpc + w
            # sort by source within each (core, window) bucket: consecutive
            # gather descriptors then walk the table monotonically, which is
            # much friendlier to HBM banks than random order
            order = np.lexsort((relidx[L][m], key))
            kcnt = np.bincount(key, minlength=n_cores * wpc)
            kst = np.r_[0, np.cumsum(kcnt)[:-1]]
            rank = np.empty(len(key), np.int64)
            rank[order] = np.arange(len(key)) - kst[key[order]]
            flat = (cb[w] + rank // P) * P + rank % P
            idxf[c, flat] = relidx[L][m].astype(np.int16)
            dsf[c, flat] = pos_of[adst[m]].astype(np.float32)

            ng = [int(k[a:b].sum()) for (a, b) in groups]
            nchg[(L, h)] = ng
            cols = sum(9 * n for n in ng)
            pk = np.zeros((n_cores, P, cols), np.int16)
            dsb = dsf.astype(ml_dtypes.bfloat16).view(np.int16)
            co = 0
            for gi, (a, b) in enumerate(groups):
                n = ng[gi]
                if n == 0:
                    continue
                s0, s1 = int(cb[a]) * P, int(cb[b]) * P
                w16 = idxf[:, s0:s1].reshape(n_cores, -1, 16).transpose(0, 2, 1)
                pk[:, :, co:co + 8 * n] = np.tile(w16, (1, 8, 1))
                pk[:, :, co + 8 * n:co + 9 * n] = \
                    dsf[:, s0:s1].astype(ml_dtypes.bfloat16).view(np.int16) \
                       .reshape(n_cores, n, P).transpose(0, 2, 1)
                co += 9 * n
            packed[(L, h)] = np.ascontiguousarray(pk)

    # per-core window constants
    nod = nodes_of.reshape(n_cores, wpc, P)
    Dw = np.broadcast_to(dinv_ext[nod].reshape(n_cores, 1, spc),
                         (n_cores, P, spc)).astype(np.float32)
    dinvw = dinv_ext[nod].transpose(0, 2, 1).astype(np.float32)  # [c,128,wpc]
    sqdegr = sqdeg_ext[nod].reshape(n_cores, 1, spc).astype(np.float32)

    # precast dinv columns per half: [128, ntiles]
    dpre = np.zeros((2, P, ntile), np.float32)
    for h in (0, 1):
        base_r = h * half_x
        nrows = (N - half_x) if h else half_x
        idx = base_r + np.arange(ntile * P) % (ntile * P)
        v = np.zeros(ntile * P, np.float32)
        v[:nrows] = dinv[base_r:base_r + nrows]
        dpre[h] = v.reshape(ntile, P).T

    nchmax = max(max(nchg[(L, h)]) for L in (1, 2) for h in (0, 1))
    pre = {
        "NCHMAX": nchmax,
        "N": N, "half_x": half_x, "n_cores": n_cores, "wpc": wpc, "spc": spc,
        "half_cc": half_cc, "ws": ws, "sstart": sstart, "outbase": outbase,
        "groups": groups, "kv": kv, "nchg": nchg, "packed": packed,
        "Dw": Dw, "dinvw": dinvw, "sqdegr": sqdegr, "dpre": dpre,
        "core_of": core_of, "slot_in_core": slot_in_core,
        "tot_rows": tot_rows,
    }
    return pre


# ------------------------------------------------------------------- builder
def _build(pre, D, H):
    N = pre["N"]
    half_x = pre["half_x"]
    n_cores = pre["n_cores"]
    wpc = pre["wpc"]
    spc = pre["spc"]
    half_cc = pre["half_cc"]
    ws = pre["ws"]
    sstart = list(pre["sstart"])
    outbase = list(pre["outbase"])
    groups = pre["groups"]
    kv = pre["kv"]
    nchg = pre["nchg"]
    JH = H // P
    f32 = mybir.dt.float32
    gdt = mybir.dt.bfloat16
    i16 = mybir.dt.int16
    half_rows = ((half_x + P - 1) // P) * P
    ntile = half_rows // P

    NCHMAX = pre["NCHMAX"]

    nc = bacc.Bacc("TRN2", target_bir_lowering=False, debug=False,
                   num_devices=n_cores, num_swdge_queues=4)
    qctr = [0]

    def next_q():
        q = qctr[0] % 4
        qctr[0] += 1
        return q

    x = nc.dram_tensor("x_all", [N, D], f32, kind="ExternalInput").ap()
    W1 = nc.dram_tensor("W1", [D, H], f32, kind="ExternalInput").ap()
    b1c = nc.dram_tensor("b1c", [P, JH], f32, kind="ExternalInput").ap()
    W2 = nc.dram_tensor("W2", [H, D], f32, kind="ExternalInput").ap()
    b2r = nc.dram_tensor("b2r", [1, D], f32, kind="ExternalInput").ap()
    iota_in = nc.dram_tensor("iota_in", [P, P], gdt, kind="ExternalInput").ap()
    pk = {}
    for L in (1, 2):
        for h in (0, 1):
            cols = pre["packed"][(L, h)].shape[2]
            pk[(L, h)] = nc.dram_tensor(f"pk{L}{h}", [P, max(cols, 1)], i16,
                                        kind="ExternalInput").ap()
    Dw_d = nc.dram_tensor("Dw", [P, spc], f32, kind="ExternalInput").ap()
    dinvw_d = nc.dram_tensor("dinvw", [P, wpc], f32, kind="ExternalInput").ap()
    dpre0_d = nc.dram_tensor("dpre0", [P, ntile], f32, kind="ExternalInput").ap()
    dpre1_d = nc.dram_tensor("dpre1", [P, ntile], f32, kind="ExternalInput").ap()
    z_out = nc.dram_tensor("z_out", [spc, D], f32, kind="ExternalOutput").ap()

    with tile.TileContext(nc) as tc:
        with tc.tile_pool(name="const", bufs=1) as cst, \
             tc.tile_pool(name="pc", bufs=3) as pc, \
             tc.tile_pool(name="tb", bufs=5) as tbp, \
             tc.tile_pool(name="gg", bufs=4) as gg, \
             tc.tile_pool(name="sel", bufs=4) as sel, \
             tc.tile_pool(name="wk", bufs=3) as wk, \
             tc.tile_pool(name="psA", bufs=2, space="PSUM") as psA, \
             tc.tile_pool(name="psB", bufs=3, space="PSUM") as psB, \
             tc.tile_pool(name="dram", bufs=1, space="DRAM") as dram:

            # ---------------- constants (f32 staging via the recycled pc pool)
            SGK = 12
            W1f = pc.tile([P, SGK * P], f32, tag="xt", name="w1f")
            nc.sync.dma_start(out=W1f[:, :H], in_=W1)
            W1b = cst.tile([P, H], gdt)
            nc.vector.tensor_copy(out=W1b[:], in_=W1f[:, :H])
            W2f = pc.tile([P, SGK * P], f32, tag="xt", name="w2f")
            for j in range(JH):
                nc.sync.dma_start(out=W2f[:, j * D:(j + 1) * D],
                                  in_=W2[j * P:(j + 1) * P, :])
            W2b = cst.tile([P, JH * D], gdt)
            nc.vector.tensor_copy(out=W2b[:], in_=W2f[:, :JH * D])
            b1_sb = cst.tile([P, JH], f32)
            nc.sync.dma_start(out=b1_sb[:], in_=b1c)
            b2f = cst.tile([1, D], f32)
            nc.sync.dma_start(out=b2f[:], in_=b2r)
            b2b = cst.tile([1, D], gdt)
            nc.vector.tensor_copy(out=b2b[:], in_=b2f[:])
            iota_sb = cst.tile([P, P], gdt)
            nc.sync.dma_start(out=iota_sb[:], in_=iota_in)
            Dw_f = pc.tile([P, SGK * P], f32, tag="xt", name="dwf")
            Dw_sb = cst.tile([P, spc], gdt)
            for j in range(0, wpc, SGK):
                k = min(SGK, wpc - j)
                nc.sync.dma_start(out=Dw_f[:, :k * P],
                                  in_=Dw_d[:, j * P:(j + k) * P])
                nc.vector.tensor_copy(out=Dw_sb[:, j * P:(j + k) * P],
                                      in_=Dw_f[:, :k * P])
            dinvw_sb = cst.tile([P, wpc], f32)
            nc.sync.dma_start(out=dinvw_sb[:], in_=dinvw_d)
            ones1 = cst.tile([1, P], gdt)
            nc.vector.memset(ones1[:], 1.0)
            b2rep = cst.tile([P, P], f32)
            ps_b2 = psB.tile([P, P], f32, tag="pu", name="psb2")
            nc.tensor.matmul(ps_b2[:], lhsT=ones1[:], rhs=b2b[:],
                             start=True, stop=True)
            nc.vector.tensor_copy(out=b2rep[:], in_=ps_b2[:])
            dpre_sb = [cst.tile([P, ntile], f32, tag=f"dp{h}", name=f"dpre{h}")
                       for h in (0, 1)]
            nc.sync.dma_start(out=dpre_sb[0][:], in_=dpre0_d)
            nc.sync.dma_start(out=dpre_sb[1][:], in_=dpre1_d)

            xg = [dram.tile([half_rows, D], gdt, tag=f"xg{h}", name=f"xg{h}")
                  for h in (0, 1)]
            ccin = [dram.tile([ws[s] * P, D], gdt, tag=f"cci{s}", name=f"cci{s}")
                    for s in range(NSLAB)]
            ccout = [dram.tile([n_cores * ws[s] * P, D], gdt, tag=f"cco{s}",
                               name=f"cco{s}", addr_space="Shared")
                     for s in range(NSLAB)]
            cc = dram.tile([pre["tot_rows"], D], gdt)

            # ---------------- precast: xg[h] = bf16(dinv * x), per half
            sgi = [0]
            for h in (0, 1):
                base_r = h * half_x
                nrows = (N - half_x) if h else half_x
                sgs = []
                r = 0
                while r + SGK * P <= nrows:
                    sgs.append((r, SGK, P))
                    r += SGK * P
                if nrows - r >= P:
                    k = (nrows - r) // P
                    sgs.append((r, k, P))
                    r += k * P
                if nrows - r > 0:
                    sgs.append((r, 1, nrows - r))
                xgv = xg[h][:].rearrange("(p t) f -> p t f", t=ntile)
                for (r0, k, prow) in sgs:
                    xt = pc.tile([P, SGK * P], f32, tag="xt")
                    xb = pc.tile([P, SGK * P], gdt, tag="xb")
                    src_ap = x[base_r + r0: base_r + r0 + (k - 1) * P + prow, :]
                    nc.scalar.dma_start(
                        out=xt[:prow, :k * P].rearrange("p (k f) -> p k f", f=P),
                        in_=src_ap.rearrange("(k p) f -> p k f", p=P) if prow == P
                        else src_ap.rearrange("(k p) f -> p k f", p=prow))
                    t0 = r0 // P
                    dv = dpre_sb[h][:prow, t0:t0 + k].unsqueeze(2) \
                        .broadcast_to([prow, k, P])
                    eng = nc.vector
                    sgi[0] += 1
                    eng.tensor_tensor(
                        out=xb[:prow, :k * P].rearrange("p (k f) -> p k f", f=P),
                        in0=xt[:prow, :k * P].rearrange("p (k f) -> p k f", f=P),
                        in1=dv, op=mybir.AluOpType.mult)
                    nc.sync.dma_start(
                        out=xgv[:prow, t0:t0 + k, :],
                        in_=xb[:prow, :k * P].rearrange("p (k f) -> p k f", f=P))

            # ---------------- aggregation sweep
            def agg_phase(L, in_aps, emit, group_end=None):
                cb = {h: np.r_[0, np.cumsum(kv[(L, h)])] for h in (0, 1)}
                coff = {h: 0 for h in (0, 1)}
                for gi, (a, b) in enumerate(groups):
                    Gs, Sels = {}, {}
                    for h in (0, 1):
                        n = nchg[(L, h)][gi]
                        if n == 0:
                            continue
                        tbt = tbp.tile([P, 9 * NCHMAX], i16, tag=f"tb{h}")
                        nc.sync.dma_start(
                            out=tbt[:, :9 * n],
                            in_=pk[(L, h)][:, coff[h]:coff[h] + 9 * n])
                        coff[h] += 9 * n
                        G = gg.tile([P, NCHMAX * P], gdt, tag=f"g{h}")
                        ca = max(1, n // 2)
                        for (c_lo, c_hi) in ((0, ca), (ca, n)):
                            if c_hi <= c_lo:
                                continue
                            nn = (c_hi - c_lo) * P
                            nc.gpsimd.dma_gather(
                                out_ap=G[:, c_lo * P:c_hi * P]
                                    .rearrange("p (k d) -> p k d", d=P),
                                in_ap=in_aps[h],
                                idxs_ap=tbt[:, c_lo * 8:c_hi * 8],
                                num_idxs=nn, num_idxs_reg=nn, elem_size=P,
                                single_packet=False, queue_num=next_q())
                        S = sel.tile([P, NCHMAX * P], gdt, tag=f"s{h}")
                        s3 = S[:, :n * P].rearrange("p (c j) -> p c j", j=P)
                        ds_ap = tbt[:, 8 * n:9 * n].bitcast(gdt)
                        d_b = ds_ap.unsqueeze(2).broadcast_to([P, n, P])
                        i_b = iota_sb[:, :P].unsqueeze(1).broadcast_to([P, n, P])
                        nc.vector.tensor_tensor(out=s3, in0=d_b, in1=i_b,
                                                op=mybir.AluOpType.is_equal)
                        Gs[h], Sels[h] = G, S
                    for w in range(a, b):
                        psum = psB.tile([P, P], f32, tag="pu")
                        ci = 0
                        for h in (0, 1):
                            if nchg[(L, h)][gi] == 0:
                                continue
                            k0 = int(cb[h][w] - cb[h][a])
                            for k in range(int(kv[(L, h)][w])):
                                off = (k0 + k) * P
                                gsl = Gs[h][:, off:off + P]
                                ssl = Sels[h][:, off:off + P]
                                if L == 1:
                                    nc.tensor.matmul(psum[:], lhsT=gsl, rhs=ssl,
                                                     start=(ci == 0), stop=False)
                                else:
                                    nc.tensor.matmul(psum[:], lhsT=ssl, rhs=gsl,
                                                     start=(ci == 0), stop=False)
                                ci += 1
                        emit(w, psum)
                    if group_end is not None:
                        group_end(gi, a, b)

            # ---- phase 1
            def emit_l1(w, psum_u):
                u_sb = wk.tile([P, P], gdt, tag="u")
                nc.vector.tensor_tensor(out=u_sb[:], in0=psum_u[:],
                                        in1=Dw_sb[:, w * P:(w + 1) * P],
                                        op=mybir.AluOpType.mult)
                psz1 = psA.tile([P, H], f32, tag="pz1")
                z1 = wk.tile([P, H], gdt, tag="z1")
                for j in range(JH):
                    nc.tensor.matmul(psz1[:, j * P:(j + 1) * P],
                                     lhsT=W1b[:, j * P:(j + 1) * P],
                                     rhs=u_sb[:], start=True, stop=True)
                    nc.scalar.activation(out=z1[:, j * P:(j + 1) * P],
                                         in_=psz1[:, j * P:(j + 1) * P],
                                         func=mybir.ActivationFunctionType.Relu,
                                         bias=b1_sb[:, j:j + 1])
                pst = psB.tile([P, P], f32, tag="pt")
                for j in range(JH):
                    nc.tensor.matmul(pst[:],
                                     lhsT=z1[:, j * P:(j + 1) * P],
                                     rhs=W2b[:, j * D:(j + 1) * D],
                                     start=(j == 0), stop=(j == JH - 1))
                t_sb = wk.tile([P, D], gdt, tag="t")
                nc.scalar.activation(out=t_sb[:], in_=pst[:],
                                     func=mybir.ActivationFunctionType.Copy,
                                     scale=dinvw_sb[:, w:w + 1])
                s = int(np.searchsorted(np.r_[sstart[1:], wpc], w, side="right"))
                wl = w - sstart[s]
                nc.scalar.dma_start(out=ccin[s][wl * P:(wl + 1) * P, :],
                                    in_=t_sb[:])
                if w == sstart[s] + ws[s] - 1:
                    nc.gpsimd.collective_compute(
                        "AllGather", mybir.AluOpType.bypass,
                        replica_groups=[list(range(n_cores))],
                        ins=[ccin[s][:]], outs=[ccout[s][:]])
                    nc.scalar.dma_start(
                        out=cc[outbase[s]:outbase[s] + n_cores * ws[s] * P, :],
                        in_=ccout[s][:])

            agg_phase(1, [xg[0][:], xg[1][:]], emit_l1)

            # ---- phase 2
            zg_box = {}

            def emit_l2(w, psum_z):
                gi = w // GROUP_W
                a = groups[gi][0]
                if w == a:
                    zg_box["t"] = wk.tile([P, GROUP_W * P], f32, tag="zg",
                                          name="zg")
                nc.vector.scalar_tensor_tensor(
                    out=zg_box["t"][:, (w - a) * P:(w - a + 1) * P],
                    in0=psum_z[:], scalar=dinvw_sb[:, w:w + 1], in1=b2rep[:],
                    op0=mybir.AluOpType.mult, op1=mybir.AluOpType.add)

            def group_end_l2(gi, a, b):
                zg = zg_box["t"]
                nc.scalar.dma_start(
                    out=z_out[a * P:b * P, :].rearrange("(w p) f -> p w f", p=P),
                    in_=zg[:, :(b - a) * P].rearrange("p (w f) -> p w f", f=P))

            agg_phase(2, [cc[0:half_cc, :], cc[half_cc:, :]], emit_l2,
                      group_end=group_end_l2)

    nc.compile()
    return nc


# -------------------------------------------------------------------- kernel
def kernel(x_all, W1, b1, W2, b2, edge_index, ix=0, max_iter=10):
    x_all = np.ascontiguousarray(np.asarray(x_all, dtype=np.float32))
    W1 = np.ascontiguousarray(np.asarray(W1, dtype=np.float32))
    b1 = np.ascontiguousarray(np.asarray(b1, dtype=np.float32))
    W2 = np.ascontiguousarray(np.asarray(W2, dtype=np.float32))
    b2 = np.ascontiguousarray(np.asarray(b2, dtype=np.float32))
    edge_index = np.asarray(edge_index)

    N, D = x_all.shape
    H = W1.shape[1]
    ekey = (N, D, H, edge_index.shape[1], GROUP_W,
            int(edge_index[0, 0]), int(edge_index[1, -1]))
    if ekey in _CACHE:
        nc, pre = _CACHE[ekey]
    else:
        pre = _preprocess(edge_index, N)
        nc = _build(pre, D, H)
        _CACHE[ekey] = (nc, pre)

    JH = H // P
    b1c = b1.reshape(JH, P).T.copy()
    b2r = b2.reshape(1, D).copy()
    iota = np.ascontiguousarray(
        np.broadcast_to(np.arange(P, dtype=np.float32)[None, None, :],
                        (P, pre["NCHMAX"], P)).reshape(P, -1)) \
        .astype(ml_dtypes.bfloat16)

    in_maps = []
    for c in range(pre["n_cores"]):
        im = {"x_all": x_all, "W1": W1, "b1c": b1c, "W2": W2, "b2r": b2r,
              "iota_in": iota,
              "Dw": pre["Dw"][c], "dinvw": pre["dinvw"][c],
              "dpre0": pre["dpre"][0], "dpre1": pre["dpre"][1]}
        for L in (1, 2):
            for h in (0, 1):
                arr = pre["packed"][(L, h)][c]
                if arr.shape[1] == 0:
                    arr = np.zeros((P, 1), np.int16)
                im[f"pk{L}{h}"] = arr
        in_maps.append(im)

    res = run_bass_kernel_spmd(nc, in_maps, core_ids=list(range(pre["n_cores"])),
                               trace=TRACE)
    LAST["exec_time_ns"] = res.exec_time_ns
    LAST["mean_exec_time_ns"] = res.mean_exec_time_ns
    LAST["per_core_scope_times"] = res.per_core_scope_times
    LAST["trace_path"] = (res.instructions_and_trace or (None, None))[1]
    LAST["profile_json"] = res.profile_json

    zs = np.stack([res.results[c]["z_out"] for c in range(pre["n_cores"])])
    z = zs[pre["core_of"], pre["slot_in_core"]]
    return z.astype(np.float32)


if __name__ == "__main__":
    rng = np.random.default_rng(0)
    N, E, D, H = 4096, 40000, 128, 512
    ei = rng.integers(0, N, size=(2, E)).astype(np.int64)
    x = rng.standard_normal((N, D), dtype=np.float32)
    W1 = rng.standard_normal((D, H), dtype=np.float32) / np.sqrt(D)
    b1 = rng.standard_normal(H).astype(np.float32) * 0.1
    W2 = rng.standard_normal((H, D), dtype=np.float32) / np.sqrt(H)
    b2 = rng.standard_normal(D).astype(np.float32) * 0.1

    deg = np.bincount(ei[1], minlength=N) + 1.0
    dinv = 1.0 / np.sqrt(deg)
    asrc = np.concatenate([ei[0], np.arange(N)])
    adst = np.concatenate([ei[1], np.arange(N)])
    nrm = dinv[asrc] * dinv[adst]

    def agg(t):
        out = np.zeros_like(t)
        np.add.at(out, adst, t[asrc] * nrm[:, None])
        return out

    z1 = np.maximum(agg(x.astype(np.float64)) @ W1 + b1, 0)
    ref = agg(z1 @ W2) + b2

    got = kernel(x, W1, b1, W2, b2, ei)
    err = np.abs(got - ref)
    rel = err.max() / np.abs(ref).max()
    print(f"exec_time_ns: {LAST['exec_time_ns']}")
    print(f"max abs err {err.max():.3e}  rel(absmax) {rel:.3e}")


# revision 40
# speedup vs baseline: 1.1314x; 1.0124x over previous
"""GCN (EAConv) 2-layer kernel for Trainium2, 8 NeuronCores — v2.

Math: z = A @ relu((A @ x) @ W1 + b1) @ W2 + b2, A = D^-1/2 (Adj+I) D^-1/2.
Factorized normalization: w_e = dinv[src]*dinv[dst] is never materialized per
edge. dinv[src] is folded into the gather tables (x pre-scaled during the
bf16 precast; t scaled when written), dinv[dst] is applied per window on the
output side (column scale for layer 1, per-partition ACT scale for layer 2,
with the bias pre-multiplied by sqrt(deg) so it survives the scale).
Selection matrices are therefore pure one-hots: ONE DVE is_equal per
(group, half) instead of two tensor_tensor passes.

Sharding: destination nodes -> 8 cores x wpc windows of 128 slots, greedy-
balanced on per-window in-edge counts split by source half. Per-core windows
are sorted by in-edge count so the shared (SPMD) per-window chunk counts
kv[w] = max over cores track each core's true need (variable chunks, ~10%
fewer than a global K cap). Weights replicated. The intermediate t = z1@W2
is exchanged with NSLAB chunked AllGathers issued as window slabs complete,
overlapping the collective under phase-1 compute.

Aggregation: one batched gpsimd.dma_gather per (group, half) pulls source
rows (edge-slot order) into G; DVE builds one-hot Sel via broadcast
is_equal; PE accumulates Sel/G chunk matmuls into PSUM per window. Dense
transforms run in bf16 on the PE.

Host-side preprocessing touches ONLY edge_index (graph structure): degrees,
node->slot permutation, edge->slot packing, int16 index tables, dinv/sqdeg
vectors. All math on x_all/W1/b1/W2/b2 runs on device.
"""
import os
import sys
import math

for _p in ("/opt/trn_rl_repo", "/root/.axon_site/_ro/trn_rl_repo"):
    if os.path.isdir(_p) and _p not in sys.path:
        sys.path.insert(0, _p)

import numpy as np
import ml_dtypes

import concourse.bass as bass
import concourse.bacc as bacc
import concourse.tile as tile
from concourse import mybir
from concourse.bass_utils import run_bass_kernel_spmd

P = 128
N_CORES = 8
GROUP_W = 3          # windows per gather group
NSLAB = 4            # chunked AllGather slabs

TRACE = False
LAST = {}            # stats from last run (exec_time_ns etc.)
_CACHE = {}


# ---------------------------------------------------------------- preprocess
def _preprocess(edge_index, n_nodes, n_cores=N_CORES):
    src = np.asarray(edge_index[0]).astype(np.int64)
    dst = np.asarray(edge_index[1]).astype(np.int64)
    N = n_nodes
    half_x = N // 2
    deg = np.bincount(dst, minlength=N).astype(np.float64) + 1.0
    dinv = (1.0 / np.sqrt(deg)).astype(np.float32)
    sqdeg = np.sqrt(deg).astype(np.float32)
    loop = np.arange(N, dtype=np.int64)
    asrc = np.concatenate([src, loop])
    adst = np.concatenate([dst, loop])
    eh1 = (asrc >= half_x).astype(np.int64)       # layer-1 table half

    w0 = np.bincount(adst[eh1 == 0], minlength=N)
    w1 = np.bincount(adst[eh1 == 1], minlength=N)

    wpc = int(math.ceil(N / n_cores / P))
    spc = wpc * P
    nwin_half = (n_cores // 2) * wpc
    assert nwin_half * P >= half_x and nwin_half * P >= (N - half_x)

    # greedy balance nodes into global windows (within x-half so layer-1
    # source halves stay balanced per window)
    win_of = np.empty(N, np.int64)
    pos_of = np.empty(N, np.int64)
    for h in (0, 1):
        nodes = np.nonzero((np.arange(N) >= half_x) == bool(h))[0]
        order = nodes[np.argsort(-(w0[nodes] + w1[nodes]), kind="stable")]
        s0 = np.zeros(nwin_half)
        s1 = np.zeros(nwin_half)
        cnt = np.zeros(nwin_half, np.int64)
        for n in order:
            score = np.maximum(s0 + w0[n], s1 + w1[n])
            score[cnt >= P] = np.inf
            b = int(np.argmin(score))
            win_of[n] = h * nwin_half + b
            pos_of[n] = cnt[b]
            cnt[b] += 1
            s0[b] += w0[n]
            s1[b] += w1[n]

    core_of = win_of // wpc
    lw_of = win_of % wpc                     # pre-perm local window

    # per-core window permutation: sort local windows by in-edge count desc
    tot = np.zeros((n_cores, wpc), np.int64)
    np.add.at(tot, (core_of[adst], lw_of[adst]), 1)
    nw_map = np.empty((n_cores, wpc), np.int64)
    for c in range(n_cores):
        order = np.argsort(-tot[c], kind="stable")
        nw_map[c, order] = np.arange(wpc)
    nw_of = nw_map[core_of, lw_of]           # sorted local window index
    slot_in_core = nw_of * P + pos_of

    # inverse map (core, nw, pos) -> node (sentinel N for empty slots)
    nodes_of = np.full((n_cores, spc), N, np.int64)
    nodes_of[core_of, slot_in_core] = np.arange(N)
    dinv_ext = np.r_[dinv, np.float32(0.0)]
    sqdeg_ext = np.r_[sqdeg, np.float32(0.0)]

    # slab partition of windows for the chunked AllGather. Geometry rule:
    # early slabs big (their collectives fire early and finish before the
    # next trigger's data is ready, so mid-phase triggers never block the
    # gpsimd gather stream), late slabs small (the phase-2-gating tail is
    # short).
    if wpc >= NSLAB * 2:
        w4 = max(1, wpc // 24)
        w3 = max(1, (wpc - w4) * 2 // 9)
        w12 = wpc - w3 - w4
        ws = [(w12 + 1) // 2, w12 // 2, w3, w4]
    else:
        base, rem = wpc // NSLAB, wpc % NSLAB
        ws = [base + (1 if i < rem else 0) for i in range(NSLAB)]
    sstart = np.r_[0, np.cumsum(ws)[:-1]].astype(np.int64)
    outbase = np.r_[0, np.cumsum([n_cores * w * P for w in ws])[:-1]].astype(np.int64)
    slab_of_w = np.repeat(np.arange(NSLAB), ws)

    # slab-major cc row of each node (layer-2 table address)
    s_n = slab_of_w[nw_of]
    ccrow = (outbase[s_n] + core_of * (np.array(ws)[s_n] * P)
             + (nw_of - sstart[s_n]) * P + pos_of)
    tot_rows = n_cores * spc
    half_cc = tot_rows // 2
    assert half_cc <= 32767 and half_x <= 32767
    eh2 = (ccrow[asrc] >= half_cc).astype(np.int64)

    # per (core, nw, half) counts for both layers
    ecore = core_of[adst]
    enw = nw_of[adst]
    def counts(eh):
        c = np.zeros((n_cores, wpc, 2), np.int64)
        np.add.at(c, (ecore, enw, eh), 1)
        return c
    c1 = counts(eh1)
    c2 = counts(eh2)
    kv = {}
    for L, c in ((1, c1), (2, c2)):
        for h in (0, 1):
            kv[(L, h)] = np.ceil(c[:, :, h].max(axis=0) / P).astype(np.int64)

    groups = []
    a = 0
    while a < wpc:
        b = min(a + GROUP_W, wpc)
        groups.append((a, b))
        a = b

    # edge -> slot tables, packed per (layer, half): per group g the columns
    # are [idx wrapped (8*nch) | ds as bf16-bits (nch)] int16
    # Layer-1 table rows are permuted to row = (n%128)*ntile + n//128 so the
    # precast can write xg with one contiguous descriptor per partition.
    half_rows = ((half_x + P - 1) // P) * P
    ntile = half_rows // P
    relx = asrc - eh1 * half_x
    relidx = {1: (relx % P) * ntile + relx // P,
              2: (ccrow[asrc] - eh2 * half_cc)}
    ehs = {1: eh1, 2: eh2}
    packed = {}
    nchg = {}
    for L in (1, 2):
        for h in (0, 1):
            k = kv[(L, h)]
            cb = np.r_[0, np.cumsum(k)]
            totch = int(cb[-1])
            idxf = np.zeros((n_cores, totch * P), np.int16)
            dsf = np.full((n_cores, totch * P), -1.0, np.float32)
            m = ehs[L] == h
            c = ecore[m]
            w = enw[m]
            key = c * wpc + w
            # sort by source within each (core, window) bucket: consecutive
            # gather descriptors then walk the table monotonically, which is
            # much friendlier to HBM banks than random order
            order = np.lexsort((relidx[L][m], key))
            kcnt = np.bincount(key, minlength=n_cores * wpc)
            kst = np.r_[0, np.cumsum(kcnt)[:-1]]
            rank = np.empty(len(key), np.int64)
            rank[order] = np.arange(len(key)) - kst[key[order]]
            flat = (cb[w] + rank // P) * P + rank % P
            idxf[c, flat] = relidx[L][m].astype(np.int16)
            dsf[c, flat] = pos_of[adst[m]].astype(np.float32)

            ng = [int(k[a:b].sum()) for (a, b) in groups]
            nchg[(L, h)] = ng
            cols = sum(9 * n for n in ng)
            pk = np.zeros((n_cores, P, cols), np.int16)
            dsb = dsf.astype(ml_dtypes.bfloat16).view(np.int16)
            co = 0
            for gi, (a, b) in enumerate(groups):
                n = ng[gi]
                if n == 0:
                    continue
                s0, s1 = int(cb[a]) * P, int(cb[b]) * P
                w16 = idxf[:, s0:s1].reshape(n_cores, -1, 16).transpose(0, 2, 1)
                pk[:, :, co:co + 8 * n] = np.tile(w16, (1, 8, 1))
                pk[:, :, co + 8 * n:co + 9 * n] = \
                    dsf[:, s0:s1].astype(ml_dtypes.bfloat16).view(np.int16) \
                       .reshape(n_cores, n, P).transpose(0, 2, 1)
                co += 9 * n
            packed[(L, h)] = np.ascontiguousarray(pk)

    # per-core window constants
    nod = nodes_of.reshape(n_cores, wpc, P)
    Dw = np.broadcast_to(dinv_ext[nod].reshape(n_cores, 1, spc),
                         (n_cores, P, spc)).astype(np.float32)
    dinvw = dinv_ext[nod].transpose(0, 2, 1).astype(np.float32)  # [c,128,wpc]
    sqdegr = sqdeg_ext[nod].reshape(n_cores, 1, spc).astype(np.float32)

    # precast dinv columns per half: [128, ntiles]
    dpre = np.zeros((2, P, ntile), np.float32)
    for h in (0, 1):
        base_r = h * half_x
        nrows = (N - half_x) if h else half_x
        idx = base_r + np.arange(ntile * P) % (ntile * P)
        v = np.zeros(ntile * P, np.float32)
        v[:nrows] = dinv[base_r:base_r + nrows]
        dpre[h] = v.reshape(ntile, P).T

    nchmax = max(max(nchg[(L, h)]) for L in (1, 2) for h in (0, 1))
    pre = {
        "NCHMAX": nchmax,
        "N": N, "half_x": half_x, "n_cores": n_cores, "wpc": wpc, "spc": spc,
        "half_cc": half_cc, "ws": ws, "sstart": sstart, "outbase": outbase,
        "groups": groups, "kv": kv, "nchg": nchg, "packed": packed,
        "Dw": Dw, "dinvw": dinvw, "sqdegr": sqdegr, "dpre": dpre,
        "core_of": core_of, "slot_in_core": slot_in_core,
        "tot_rows": tot_rows,
    }
    return pre


# ------------------------------------------------------------------- builder
def _build(pre, D, H):
    N = pre["N"]
    half_x = pre["half_x"]
    n_cores = pre["n_cores"]
    wpc = pre["wpc"]
    spc = pre["spc"]
    half_cc = pre["half_cc"]
    ws = pre["ws"]
    sstart = list(pre["sstart"])
    outbase = list(pre["outbase"])
    groups = pre["groups"]
    kv = pre["kv"]
    nchg = pre["nchg"]
    JH = H // P
    f32 = mybir.dt.float32
    gdt = mybir.dt.bfloat16
    i16 = mybir.dt.int16
    half_rows = ((half_x + P - 1) // P) * P
    ntile = half_rows // P

    NCHMAX = pre["NCHMAX"]

    nc = bacc.Bacc("TRN2", target_bir_lowering=False, debug=False,
                   num_devices=n_cores, num_swdge_queues=4)
    qctr = [0]

    def next_q():
        q = qctr[0] % 4
        qctr[0] += 1
        return q

    x = nc.dram_tensor("x_all", [N, D], f32, kind="ExternalInput").ap()
    W1 = nc.dram_tensor("W1", [D, H], f32, kind="ExternalInput").ap()
    b1c = nc.dram_tensor("b1c", [P, JH], f32, kind="ExternalInput").ap()
    W2 = nc.dram_tensor("W2", [H, D], f32, kind="ExternalInput").ap()
    b2r = nc.dram_tensor("b2r", [1, D], f32, kind="ExternalInput").ap()
    iota_in = nc.dram_tensor("iota_in", [P, P], gdt, kind="ExternalInput").ap()
    pk = {}
    for L in (1, 2):
        for h in (0, 1):
            cols = pre["packed"][(L, h)].shape[2]
            pk[(L, h)] = nc.dram_tensor(f"pk{L}{h}", [P, max(cols, 1)], i16,
                                        kind="ExternalInput").ap()
    Dw_d = nc.dram_tensor("Dw", [P, spc], f32, kind="ExternalInput").ap()
    dinvw_d = nc.dram_tensor("dinvw", [P, wpc], f32, kind="ExternalInput").ap()
    dpre0_d = nc.dram_tensor("dpre0", [P, ntile], f32, kind="ExternalInput").ap()
    dpre1_d = nc.dram_tensor("dpre1", [P, ntile], f32, kind="ExternalInput").ap()
    z_out = nc.dram_tensor("z_out", [spc, D], f32, kind="ExternalOutput").ap()

    with tile.TileContext(nc) as tc:
        with tc.tile_pool(name="const", bufs=1) as cst, \
             tc.tile_pool(name="pc", bufs=3) as pc, \
             tc.tile_pool(name="tb", bufs=5) as tbp, \
             tc.tile_pool(name="gg", bufs=4) as gg, \
             tc.tile_pool(name="sel", bufs=4) as sel, \
             tc.tile_pool(name="wk", bufs=3) as wk, \
             tc.tile_pool(name="psA", bufs=2, space="PSUM") as psA, \
             tc.tile_pool(name="psB", bufs=3, space="PSUM") as psB, \
             tc.tile_pool(name="dram", bufs=1, space="DRAM") as dram:

            # ---------------- constants (f32 staging via the recycled pc pool)
            SGK = 12
            W1f = pc.tile([P, SGK * P], f32, tag="xt", name="w1f")
            nc.sync.dma_start(out=W1f[:, :H], in_=W1)
            W1b = cst.tile([P, H], gdt)
            nc.vector.tensor_copy(out=W1b[:], in_=W1f[:, :H])
            W2f = pc.tile([P, SGK * P], f32, tag="xt", name="w2f")
            for j in range(JH):
                nc.sync.dma_start(out=W2f[:, j * D:(j + 1) * D],
                                  in_=W2[j * P:(j + 1) * P, :])
            W2b = cst.tile([P, JH * D], gdt)
            nc.vector.tensor_copy(out=W2b[:], in_=W2f[:, :JH * D])
            b1_sb = cst.tile([P, JH], f32)
            nc.sync.dma_start(out=b1_sb[:], in_=b1c)
            b2f = cst.tile([1, D], f32)
            nc.sync.dma_start(out=b2f[:], in_=b2r)
            b2b = cst.tile([1, D], gdt)
            nc.vector.tensor_copy(out=b2b[:], in_=b2f[:])
            iota_sb = cst.tile([P, P], gdt)
            nc.sync.dma_start(out=iota_sb[:], in_=iota_in)
            Dw_f = pc.tile([P, SGK * P], f32, tag="xt", name="dwf")
            Dw_sb = cst.tile([P, spc], gdt)
            for j in range(0, wpc, SGK):
                k = min(SGK, wpc - j)
                nc.sync.dma_start(out=Dw_f[:, :k * P],
                                  in_=Dw_d[:, j * P:(j + k) * P])
                nc.vector.tensor_copy(out=Dw_sb[:, j * P:(j + k) * P],
                                      in_=Dw_f[:, :k * P])
            dinvw_sb = cst.tile([P, wpc], f32)
            nc.sync.dma_start(out=dinvw_sb[:], in_=dinvw_d)
            ones1 = cst.tile([1, P], gdt)
            nc.vector.memset(ones1[:], 1.0)
            b2rep = cst.tile([P, P], f32)
            ps_b2 = psB.tile([P, P], f32, tag="pu", name="psb2")
            nc.tensor.matmul(ps_b2[:], lhsT=ones1[:], rhs=b2b[:],
                             start=True, stop=True)
            nc.vector.tensor_copy(out=b2rep[:], in_=ps_b2[:])
            dpre_sb = [cst.tile([P, ntile], f32, tag=f"dp{h}", name=f"dpre{h}")
                       for h in (0, 1)]
            nc.sync.dma_start(out=dpre_sb[0][:], in_=dpre0_d)
            nc.sync.dma_start(out=dpre_sb[1][:], in_=dpre1_d)

            xg = [dram.tile([half_rows, D], gdt, tag=f"xg{h}", name=f"xg{h}")
                  for h in (0, 1)]
            ccin = [dram.tile([ws[s] * P, D], gdt, tag=f"cci{s}", name=f"cci{s}")
                    for s in range(NSLAB)]
            ccout = [dram.tile([n_cores * ws[s] * P, D], gdt, tag=f"cco{s}",
                               name=f"cco{s}", addr_space="Shared")
                     for s in range(NSLAB)]
            cc = dram.tile([pre["tot_rows"], D], gdt)

            # ---------------- precast: xg[h] = bf16(dinv * x), per half
            sgi = [0]
            for h in (0, 1):
                base_r = h * half_x
                nrows = (N - half_x) if h else half_x
                sgs = []
                r = 0
                while r + SGK * P <= nrows:
                    sgs.append((r, SGK, P))
                    r += SGK * P
                if nrows - r >= P:
                    k = (nrows - r) // P
                    sgs.append((r, k, P))
                    r += k * P
                if nrows - r > 0:
                    sgs.append((r, 1, nrows - r))
                xgv = xg[h][:].rearrange("(p t) f -> p t f", t=ntile)
                for (r0, k, prow) in sgs:
                    xt = pc.tile([P, SGK * P], f32, tag="xt")
                    xb = pc.tile([P, SGK * P], gdt, tag="xb")
                    src_ap = x[base_r + r0: base_r + r0 + (k - 1) * P + prow, :]
                    nc.scalar.dma_start(
                        out=xt[:prow, :k * P].rearrange("p (k f) -> p k f", f=P),
                        in_=src_ap.rearrange("(k p) f -> p k f", p=P) if prow == P
                        else src_ap.rearrange("(k p) f -> p k f", p=prow))
                    t0 = r0 // P
                    dv = dpre_sb[h][:prow, t0:t0 + k].unsqueeze(2) \
                        .broadcast_to([prow, k, P])
                    eng = nc.vector
                    sgi[0] += 1
                    eng.tensor_tensor(
                        out=xb[:prow, :k * P].rearrange("p (k f) -> p k f", f=P),
                        in0=xt[:prow, :k * P].rearrange("p (k f) -> p k f", f=P),
                        in1=dv, op=mybir.AluOpType.mult)
                    nc.sync.dma_start(
                        out=xgv[:prow, t0:t0 + k, :],
                        in_=xb[:prow, :k * P].rearrange("p (k f) -> p k f", f=P))

            # ---------------- aggregation sweep
            def agg_phase(L, in_aps, emit, group_end=None):
                cb = {h: np.r_[0, np.cumsum(kv[(L, h)])] for h in (0, 1)}
                coff = {h: 0 for h in (0, 1)}
                for gi, (a, b) in enumerate(groups):
                    Gs, Sels = {}, {}
                    for h in (0, 1):
                        n = nchg[(L, h)][gi]
                        if n == 0:
                            continue
                        tbt = tbp.tile([P, 9 * NCHMAX], i16, tag=f"tb{h}")
                        nc.sync.dma_start(
                            out=tbt[:, :9 * n],
                            in_=pk[(L, h)][:, coff[h]:coff[h] + 9 * n])
                        coff[h] += 9 * n
                        G = gg.tile([P, NCHMAX * P], gdt, tag=f"g{h}")
                        ca = max(1, n // 2)
                        for (c_lo, c_hi) in ((0, ca), (ca, n)):
                            if c_hi <= c_lo:
                                continue
                            nn = (c_hi - c_lo) * P
                            nc.gpsimd.dma_gather(
                                out_ap=G[:, c_lo * P:c_hi * P]
                                    .rearrange("p (k d) -> p k d", d=P),
                                in_ap=in_aps[h],
                                idxs_ap=tbt[:, c_lo * 8:c_hi * 8],
                                num_idxs=nn, num_idxs_reg=nn, elem_size=P,
                                single_packet=False, queue_num=next_q())
                        S = sel.tile([P, NCHMAX * P], gdt, tag=f"s{h}")
                        s3 = S[:, :n * P].rearrange("p (c j) -> p c j", j=P)
                        ds_ap = tbt[:, 8 * n:9 * n].bitcast(gdt)
                        d_b = ds_ap.unsqueeze(2).broadcast_to([P, n, P])
                        i_b = iota_sb[:, :P].unsqueeze(1).broadcast_to([P, n, P])
                        nc.vector.tensor_tensor(out=s3, in0=d_b, in1=i_b,
                                                op=mybir.AluOpType.is_equal)
                        Gs[h], Sels[h] = G, S
                    for w in range(a, b):
                        psum = psB.tile([P, P], f32, tag="pu")
                        ci = 0
                        for h in (0, 1):
                            if nchg[(L, h)][gi] == 0:
                                continue
                            k0 = int(cb[h][w] - cb[h][a])
                            for k in range(int(kv[(L, h)][w])):
                                off = (k0 + k) * P
                                gsl = Gs[h][:, off:off + P]
                                ssl = Sels[h][:, off:off + P]
                                if L == 1:
                                    nc.tensor.matmul(psum[:], lhsT=gsl, rhs=ssl,
                                                     start=(ci == 0), stop=False)
                                else:
                                    nc.tensor.matmul(psum[:], lhsT=ssl, rhs=gsl,
                                                     start=(ci == 0), stop=False)
                                ci += 1
                        emit(w, psum)
                    if group_end is not None:
                        group_end(gi, a, b)

            # ---- phase 1
            def emit_l1(w, psum_u):
                u_sb = wk.tile([P, P], gdt, tag="u")
                nc.vector.tensor_tensor(out=u_sb[:], in0=psum_u[:],
                                        in1=Dw_sb[:, w * P:(w + 1) * P],
                                        op=mybir.AluOpType.mult)
                psz1 = psA.tile([P, H], f32, tag="pz1")
                z1 = wk.tile([P, H], gdt, tag="z1")
                for j in range(JH):
                    nc.tensor.matmul(psz1[:, j * P:(j + 1) * P],
                                     lhsT=W1b[:, j * P:(j + 1) * P],
                                     rhs=u_sb[:], start=True, stop=True)
                    nc.scalar.activation(out=z1[:, j * P:(j + 1) * P],
                                         in_=psz1[:, j * P:(j + 1) * P],
                                         func=mybir.ActivationFunctionType.Relu,
                                         bias=b1_sb[:, j:j + 1])
                pst = psB.tile([P, P], f32, tag="pt")
                for j in range(JH):
                    nc.tensor.matmul(pst[:],
                                     lhsT=z1[:, j * P:(j + 1) * P],
                                     rhs=W2b[:, j * D:(j + 1) * D],
                                     start=(j == 0), stop=(j == JH - 1))
                t_sb = wk.tile([P, D], gdt, tag="t")
                nc.scalar.activation(out=t_sb[:], in_=pst[:],
                                     func=mybir.ActivationFunctionType.Copy,
                                     scale=dinvw_sb[:, w:w + 1])
                s = int(np.searchsorted(np.r_[sstart[1:], wpc], w, side="right"))
                wl = w - sstart[s]
                nc.scalar.dma_start(out=ccin[s][wl * P:(wl + 1) * P, :],
                                    in_=t_sb[:])
                if w == sstart[s] + ws[s] - 1:
                    nc.gpsimd.collective_compute(
                        "AllGather", mybir.AluOpType.bypass,
                        replica_groups=[list(range(n_cores))],
                        ins=[ccin[s][:]], outs=[ccout[s][:]])
                    nc.scalar.dma_start(
                        out=cc[outbase[s]:outbase[s] + n_cores * ws[s] * P, :],
                        in_=ccout[s][:])

            agg_phase(1, [xg[0][:], xg[1][:]], emit_l1)

            # ---- phase 2
            zg_box = {}

            def emit_l2(w, psum_z):
                gi = w // GROUP_W
                a = groups[gi][0]
                if w == a:
                    zg_box["t"] = wk.tile([P, GROUP_W * P], f32, tag="zg",
                                          name="zg")
                nc.vector.scalar_tensor_tensor(
                    out=zg_box["t"][:, (w - a) * P:(w - a + 1) * P],
                    in0=psum_z[:], scalar=dinvw_sb[:, w:w + 1], in1=b2rep[:],
                    op0=mybir.AluOpType.mult, op1=mybir.AluOpType.add)

            def group_end_l2(gi, a, b):
                zg = zg_box["t"]
                nc.scalar.dma_start(
                    out=z_out[a * P:b * P, :].rearrange("(w p) f -> p w f", p=P),
                    in_=zg[:, :(b - a) * P].rearrange("p (w f) -> p w f", f=P))

            agg_phase(2, [cc[0:half_cc, :], cc[half_cc:, :]], emit_l2,
                      group_end=group_end_l2)

    nc.compile()
    return nc


# -------------------------------------------------------------------- kernel
def kernel(x_all, W1, b1, W2, b2, edge_index, ix=0, max_iter=10):
    x_all = np.ascontiguousarray(np.asarray(x_all, dtype=np.float32))
    W1 = np.ascontiguousarray(np.asarray(W1, dtype=np.float32))
    b1 = np.ascontiguousarray(np.asarray(b1, dtype=np.float32))
    W2 = np.ascontiguousarray(np.asarray(W2, dtype=np.float32))
    b2 = np.ascontiguousarray(np.asarray(b2, dtype=np.float32))
    edge_index = np.asarray(edge_index)

    N, D = x_all.shape
    H = W1.shape[1]
    ekey = (N, D, H, edge_index.shape[1], GROUP_W,
            int(edge_index[0, 0]), int(edge_index[1, -1]))
    if ekey in _CACHE:
        nc, pre = _CACHE[ekey]
    else:
        pre = _preprocess(edge_index, N)
        nc = _build(pre, D, H)
        _CACHE[ekey] = (nc, pre)

    JH = H // P
    b1c = b1.reshape(JH, P).T.copy()
    b2r = b2.reshape(1, D).copy()
    iota = np.ascontiguousarray(
        np.broadcast_to(np.arange(P, dtype=np.float32)[None, None, :],
                        (P, pre["NCHMAX"], P)).reshape(P, -1)) \
        .astype(ml_dtypes.bfloat16)

    in_maps = []
    for c in range(pre["n_cores"]):
        im = {"x_all": x_all, "W1": W1, "b1c": b1c, "W2": W2, "b2r": b2r,
              "iota_in": iota,
              "Dw": pre["Dw"][c], "dinvw": pre["dinvw"][c],
              "dpre0": pre["dpre"][0], "dpre1": pre["dpre"][1]}
        for L in (1, 2):
            for h in (0, 1):
                arr = pre["packed"][(L, h)][c]
                if arr.shape[1] == 0:
                    arr = np.zeros((P, 1), np.int16)
                im[f"pk{L}{h}"] = arr
        in_maps.append(im)

    res = run_bass_kernel_spmd(nc, in_maps, core_ids=list(range(pre["n_cores"])),
                               trace=TRACE)
    LAST["exec_time_ns"] = res.exec_time_ns
    LAST["mean_exec_time_ns"] = res.mean_exec_time_ns
    LAST["per_core_scope_times"] = res.per_core_scope_times
    LAST["trace_path"] = (res.instructions_and_trace or (None, None))[1]
    LAST["profile_json"] = res.profile_json

    zs = np.stack([res.results[c]["z_out"] for c in range(pre["n_cores"])])
    z = zs[pre["core_of"], pre["slot_in_core"]]
    return z.astype(np.float32)


if __name__ == "__main__":
    rng = np.random.default_rng(0)
    N, E, D, H = 4096, 40000, 128, 512
    ei = rng.integers(0, N, size=(2, E)).astype(np.int64)
    x = rng.standard_normal((N, D), dtype=np.float32)
    W1 = rng.standard_normal((D, H), dtype=np.float32) / np.sqrt(D)
    b1 = rng.standard_normal(H).astype(np.float32) * 0.1
    W2 = rng.standard_normal((H, D), dtype=np.float32) / np.sqrt(H)
    b2 = rng.standard_normal(D).astype(np.float32) * 0.1

    deg = np.bincount(ei[1], minlength=N) + 1.0
    dinv = 1.0 / np.sqrt(deg)
    asrc = np.concatenate([ei[0], np.arange(N)])
    adst = np.concatenate([ei[1], np.arange(N)])
    nrm = dinv[asrc] * dinv[adst]

    def agg(t):
        out = np.zeros_like(t)
        np.add.at(out, adst, t[asrc] * nrm[:, None])
        return out

    z1 = np.maximum(agg(x.astype(np.float64)) @ W1 + b1, 0)
    ref = agg(z1 @ W2) + b2

    got = kernel(x, W1, b1, W2, b2, ei)
    err = np.abs(got - ref)
    rel = err.max() / np.abs(ref).max()
    print(f"exec_time_ns: {LAST['exec_time_ns']}")
    print(f"max abs err {err.max():.3e}  rel(absmax) {rel:.3e}")


# revision 45
# speedup vs baseline: 1.1650x; 1.0297x over previous
"""GCN (EAConv) 2-layer kernel for Trainium2, 8 NeuronCores — v2.

Math: z = A @ relu((A @ x) @ W1 + b1) @ W2 + b2, A = D^-1/2 (Adj+I) D^-1/2.
Factorized normalization: w_e = dinv[src]*dinv[dst] is never materialized per
edge. dinv[src] is folded into the gather tables (x pre-scaled during the
bf16 precast; t scaled when written), dinv[dst] is applied per window on the
output side (column scale for layer 1, per-partition ACT scale for layer 2,
with the bias pre-multiplied by sqrt(deg) so it survives the scale).
Selection matrices are therefore pure one-hots: ONE DVE is_equal per
(group, half) instead of two tensor_tensor passes.

Sharding: destination nodes -> 8 cores x wpc windows of 128 slots, greedy-
balanced on per-window in-edge counts split by source half. Per-core windows
are sorted by in-edge count so the shared (SPMD) per-window chunk counts
kv[w] = max over cores track each core's true need (variable chunks, ~10%
fewer than a global K cap). Weights replicated. The intermediate t = z1@W2
is exchanged with NSLAB chunked AllGathers issued as window slabs complete,
overlapping the collective under phase-1 compute.

Aggregation: one batched gpsimd.dma_gather per (group, half) pulls source
rows (edge-slot order) into G; DVE builds one-hot Sel via broadcast
is_equal; PE accumulates Sel/G chunk matmuls into PSUM per window. Dense
transforms run in bf16 on the PE.

Host-side preprocessing touches ONLY edge_index (graph structure): degrees,
node->slot permutation, edge->slot packing, int16 index tables, dinv/sqdeg
vectors. All math on x_all/W1/b1/W2/b2 runs on device.
"""
import os
import sys
import math

for _p in ("/opt/trn_rl_repo", "/root/.axon_site/_ro/trn_rl_repo"):
    if os.path.isdir(_p) and _p not in sys.path:
        sys.path.insert(0, _p)

import numpy as np
import ml_dtypes

import concourse.bass as bass
import concourse.bacc as bacc
import concourse.tile as tile
from concourse import mybir
from concourse.bass_utils import run_bass_kernel_spmd

P = 128
N_CORES = 8
GROUP_W = 3          # windows per gather group
NSLAB = 3            # chunked AllGather slabs

TRACE = False
LAST = {}            # stats from last run (exec_time_ns etc.)
_CACHE = {}


# ---------------------------------------------------------------- preprocess
def _preprocess(edge_index, n_nodes, n_cores=N_CORES):
    src = np.asarray(edge_index[0]).astype(np.int64)
    dst = np.asarray(edge_index[1]).astype(np.int64)
    N = n_nodes
    half_x = N // 2
    deg = np.bincount(dst, minlength=N).astype(np.float64) + 1.0
    dinv = (1.0 / np.sqrt(deg)).astype(np.float32)
    sqdeg = np.sqrt(deg).astype(np.float32)
    loop = np.arange(N, dtype=np.int64)
    asrc = np.concatenate([src, loop])
    adst = np.concatenate([dst, loop])
    eh1 = (asrc >= half_x).astype(np.int64)       # layer-1 table half

    w0 = np.bincount(adst[eh1 == 0], minlength=N)
    w1 = np.bincount(adst[eh1 == 1], minlength=N)

    wpc = int(math.ceil(N / n_cores / P))
    spc = wpc * P
    nwin_half = (n_cores // 2) * wpc
    assert nwin_half * P >= half_x and nwin_half * P >= (N - half_x)

    # greedy balance nodes into global windows (within x-half so layer-1
    # source halves stay balanced per window)
    win_of = np.empty(N, np.int64)
    pos_of = np.empty(N, np.int64)
    for h in (0, 1):
        nodes = np.nonzero((np.arange(N) >= half_x) == bool(h))[0]
        order = nodes[np.argsort(-(w0[nodes] + w1[nodes]), kind="stable")]
        s0 = np.zeros(nwin_half)
        s1 = np.zeros(nwin_half)
        cnt = np.zeros(nwin_half, np.int64)
        for n in order:
            score = np.maximum(s0 + w0[n], s1 + w1[n])
            score[cnt >= P] = np.inf
            b = int(np.argmin(score))
            win_of[n] = h * nwin_half + b
            pos_of[n] = cnt[b]
            cnt[b] += 1
            s0[b] += w0[n]
            s1[b] += w1[n]

    core_of = win_of // wpc
    lw_of = win_of % wpc                     # pre-perm local window

    # per-core window permutation: sort local windows by in-edge count desc
    tot = np.zeros((n_cores, wpc), np.int64)
    np.add.at(tot, (core_of[adst], lw_of[adst]), 1)
    nw_map = np.empty((n_cores, wpc), np.int64)
    for c in range(n_cores):
        order = np.argsort(-tot[c], kind="stable")
        nw_map[c, order] = np.arange(wpc)
    nw_of = nw_map[core_of, lw_of]           # sorted local window index
    slot_in_core = nw_of * P + pos_of

    # inverse map (core, nw, pos) -> node (sentinel N for empty slots)
    nodes_of = np.full((n_cores, spc), N, np.int64)
    nodes_of[core_of, slot_in_core] = np.arange(N)
    dinv_ext = np.r_[dinv, np.float32(0.0)]
    sqdeg_ext = np.r_[sqdeg, np.float32(0.0)]

    # slab partition of windows for the chunked AllGather. Geometry rule:
    # early slabs big (their collectives fire early and finish before the
    # next trigger's data is ready, so mid-phase triggers never block the
    # gpsimd gather stream), late slabs small (the phase-2-gating tail is
    # short).
    if wpc >= NSLAB * 2:
        w3 = max(1, wpc // 24)
        w12 = wpc - w3
        ws = [(w12 + 1) // 2, w12 // 2, w3]
    else:
        base, rem = wpc // NSLAB, wpc % NSLAB
        ws = [base + (1 if i < rem else 0) for i in range(NSLAB)]
    sstart = np.r_[0, np.cumsum(ws)[:-1]].astype(np.int64)
    outbase = np.r_[0, np.cumsum([n_cores * w * P for w in ws])[:-1]].astype(np.int64)
    slab_of_w = np.repeat(np.arange(NSLAB), ws)

    # slab-major cc row of each node (layer-2 table address)
    s_n = slab_of_w[nw_of]
    ccrow = (outbase[s_n] + core_of * (np.array(ws)[s_n] * P)
             + (nw_of - sstart[s_n]) * P + pos_of)
    tot_rows = n_cores * spc
    half_cc = tot_rows // 2
    assert half_cc <= 32767 and half_x <= 32767
    eh2 = (ccrow[asrc] >= half_cc).astype(np.int64)

    # per (core, nw, half) counts for both layers
    ecore = core_of[adst]
    enw = nw_of[adst]
    def counts(eh):
        c = np.zeros((n_cores, wpc, 2), np.int64)
        np.add.at(c, (ecore, enw, eh), 1)
        return c
    c1 = counts(eh1)
    c2 = counts(eh2)
    kv = {}
    for L, c in ((1, c1), (2, c2)):
        for h in (0, 1):
            kv[(L, h)] = np.ceil(c[:, :, h].max(axis=0) / P).astype(np.int64)

    groups = []
    a = 0
    while a < wpc:
        b = min(a + GROUP_W, wpc)
        groups.append((a, b))
        a = b

    # edge -> slot tables, packed per (layer, half): per group g the columns
    # are [idx wrapped (8*nch) | ds as bf16-bits (nch)] int16
    # Layer-1 table rows are permuted to row = (n%128)*ntile + n//128 so the
    # precast can write xg with one contiguous descriptor per partition.
    half_rows = ((half_x + P - 1) // P) * P
    ntile = half_rows // P
    relx = asrc - eh1 * half_x
    relidx = {1: (relx % P) * ntile + relx // P,
              2: (ccrow[asrc] - eh2 * half_cc)}
    ehs = {1: eh1, 2: eh2}
    packed = {}
    nchg = {}
    for L in (1, 2):
        for h in (0, 1):
            k = kv[(L, h)]
            cb = np.r_[0, np.cumsum(k)]
            totch = int(cb[-1])
            idxf = np.zeros((n_cores, totch * P), np.int16)
            dsf = np.full((n_cores, totch * P), -1.0, np.float32)
            m = ehs[L] == h
            c = ecore[m]
            w = enw[m]
            key = c * wpc + w
            # sort by source within each (core, window) bucket: consecutive
            # gather descriptors then walk the table monotonically, which is
            # much friendlier to HBM banks than random order
            order = np.lexsort((relidx[L][m], key))
            kcnt = np.bincount(key, minlength=n_cores * wpc)
            kst = np.r_[0, np.cumsum(kcnt)[:-1]]
            rank = np.empty(len(key), np.int64)
            rank[order] = np.arange(len(key)) - kst[key[order]]
            flat = (cb[w] + rank // P) * P + rank % P
            idxf[c, flat] = relidx[L][m].astype(np.int16)
            dsf[c, flat] = pos_of[adst[m]].astype(np.float32)

            ng = [int(k[a:b].sum()) for (a, b) in groups]
            nchg[(L, h)] = ng
            cols = sum(9 * n for n in ng)
            pk = np.zeros((n_cores, P, cols), np.int16)
            dsb = dsf.astype(ml_dtypes.bfloat16).view(np.int16)
            co = 0
            for gi, (a, b) in enumerate(groups):
                n = ng[gi]
                if n == 0:
                    continue
                s0, s1 = int(cb[a]) * P, int(cb[b]) * P
                w16 = idxf[:, s0:s1].reshape(n_cores, -1, 16).transpose(0, 2, 1)
                pk[:, :, co:co + 8 * n] = np.tile(w16, (1, 8, 1))
                pk[:, :, co + 8 * n:co + 9 * n] = \
                    dsf[:, s0:s1].astype(ml_dtypes.bfloat16).view(np.int16) \
                       .reshape(n_cores, n, P).transpose(0, 2, 1)
                co += 9 * n
            packed[(L, h)] = np.ascontiguousarray(pk)

    # per-core window constants
    nod = nodes_of.reshape(n_cores, wpc, P)
    Dw = np.broadcast_to(dinv_ext[nod].reshape(n_cores, 1, spc),
                         (n_cores, P, spc)).astype(np.float32)
    dinvw = dinv_ext[nod].transpose(0, 2, 1).astype(np.float32)  # [c,128,wpc]
    sqdegr = sqdeg_ext[nod].reshape(n_cores, 1, spc).astype(np.float32)

    # precast dinv columns per half: [128, ntiles]
    dpre = np.zeros((2, P, ntile), np.float32)
    for h in (0, 1):
        base_r = h * half_x
        nrows = (N - half_x) if h else half_x
        idx = base_r + np.arange(ntile * P) % (ntile * P)
        v = np.zeros(ntile * P, np.float32)
        v[:nrows] = dinv[base_r:base_r + nrows]
        dpre[h] = v.reshape(ntile, P).T

    nchmax = max(max(nchg[(L, h)]) for L in (1, 2) for h in (0, 1))
    pre = {
        "NCHMAX": nchmax,
        "N": N, "half_x": half_x, "n_cores": n_cores, "wpc": wpc, "spc": spc,
        "half_cc": half_cc, "ws": ws, "sstart": sstart, "outbase": outbase,
        "groups": groups, "kv": kv, "nchg": nchg, "packed": packed,
        "Dw": Dw, "dinvw": dinvw, "sqdegr": sqdegr, "dpre": dpre,
        "core_of": core_of, "slot_in_core": slot_in_core,
        "tot_rows": tot_rows,
    }
    return pre


# ------------------------------------------------------------------- builder
def _build(pre, D, H):
    N = pre["N"]
    half_x = pre["half_x"]
    n_cores = pre["n_cores"]
    wpc = pre["wpc"]
    spc = pre["spc"]
    half_cc = pre["half_cc"]
    ws = pre["ws"]
    sstart = list(pre["sstart"])
    outbase = list(pre["outbase"])
    groups = pre["groups"]
    kv = pre["kv"]
    nchg = pre["nchg"]
    JH = H // P
    f32 = mybir.dt.float32
    gdt = mybir.dt.bfloat16
    i16 = mybir.dt.int16
    half_rows = ((half_x + P - 1) // P) * P
    ntile = half_rows // P

    NCHMAX = pre["NCHMAX"]

    nc = bacc.Bacc("TRN2", target_bir_lowering=False, debug=False,
                   num_devices=n_cores, num_swdge_queues=4)
    qctr = [0]

    def next_q():
        q = qctr[0] % 4
        qctr[0] += 1
        return q

    x = nc.dram_tensor("x_all", [N, D], f32, kind="ExternalInput").ap()
    W1 = nc.dram_tensor("W1", [D, H], f32, kind="ExternalInput").ap()
    b1c = nc.dram_tensor("b1c", [P, JH], f32, kind="ExternalInput").ap()
    W2 = nc.dram_tensor("W2", [H, D], f32, kind="ExternalInput").ap()
    b2r = nc.dram_tensor("b2r", [1, D], f32, kind="ExternalInput").ap()
    iota_in = nc.dram_tensor("iota_in", [P, P], gdt, kind="ExternalInput").ap()
    pk = {}
    for L in (1, 2):
        for h in (0, 1):
            cols = pre["packed"][(L, h)].shape[2]
            pk[(L, h)] = nc.dram_tensor(f"pk{L}{h}", [P, max(cols, 1)], i16,
                                        kind="ExternalInput").ap()
    Dw_d = nc.dram_tensor("Dw", [P, spc], f32, kind="ExternalInput").ap()
    dinvw_d = nc.dram_tensor("dinvw", [P, wpc], f32, kind="ExternalInput").ap()
    dpre0_d = nc.dram_tensor("dpre0", [P, ntile], f32, kind="ExternalInput").ap()
    dpre1_d = nc.dram_tensor("dpre1", [P, ntile], f32, kind="ExternalInput").ap()
    z_out = nc.dram_tensor("z_out", [spc, D], f32, kind="ExternalOutput").ap()

    with tile.TileContext(nc) as tc:
        with tc.tile_pool(name="const", bufs=1) as cst, \
             tc.tile_pool(name="pc", bufs=3) as pc, \
             tc.tile_pool(name="tb", bufs=5) as tbp, \
             tc.tile_pool(name="gg", bufs=4) as gg, \
             tc.tile_pool(name="sel", bufs=4) as sel, \
             tc.tile_pool(name="wk", bufs=3) as wk, \
             tc.tile_pool(name="psA", bufs=2, space="PSUM") as psA, \
             tc.tile_pool(name="psB", bufs=3, space="PSUM") as psB, \
             tc.tile_pool(name="dram", bufs=1, space="DRAM") as dram:

            # ---------------- constants (f32 staging via the recycled pc pool)
            SGK = 12
            W1f = pc.tile([P, SGK * P], f32, tag="xt", name="w1f")
            nc.sync.dma_start(out=W1f[:, :H], in_=W1)
            W1b = cst.tile([P, H], gdt)
            nc.vector.tensor_copy(out=W1b[:], in_=W1f[:, :H])
            W2f = pc.tile([P, SGK * P], f32, tag="xt", name="w2f")
            for j in range(JH):
                nc.sync.dma_start(out=W2f[:, j * D:(j + 1) * D],
                                  in_=W2[j * P:(j + 1) * P, :])
            W2b = cst.tile([P, JH * D], gdt)
            nc.vector.tensor_copy(out=W2b[:], in_=W2f[:, :JH * D])
            b1_sb = cst.tile([P, JH], f32)
            nc.sync.dma_start(out=b1_sb[:], in_=b1c)
            b2f = cst.tile([1, D], f32)
            nc.sync.dma_start(out=b2f[:], in_=b2r)
            b2b = cst.tile([1, D], gdt)
            nc.vector.tensor_copy(out=b2b[:], in_=b2f[:])
            iota_sb = cst.tile([P, P], gdt)
            nc.sync.dma_start(out=iota_sb[:], in_=iota_in)
            Dw_f = pc.tile([P, SGK * P], f32, tag="xt", name="dwf")
            Dw_sb = cst.tile([P, spc], gdt)
            for j in range(0, wpc, SGK):
                k = min(SGK, wpc - j)
                nc.sync.dma_start(out=Dw_f[:, :k * P],
                                  in_=Dw_d[:, j * P:(j + k) * P])
                nc.vector.tensor_copy(out=Dw_sb[:, j * P:(j + k) * P],
                                      in_=Dw_f[:, :k * P])
            dinvw_sb = cst.tile([P, wpc], f32)
            nc.sync.dma_start(out=dinvw_sb[:], in_=dinvw_d)
            ones1 = cst.tile([1, P], gdt)
            nc.vector.memset(ones1[:], 1.0)
            b2rep = cst.tile([P, P], f32)
            ps_b2 = psB.tile([P, P], f32, tag="pu", name="psb2")
            nc.tensor.matmul(ps_b2[:], lhsT=ones1[:], rhs=b2b[:],
                             start=True, stop=True)
            nc.vector.tensor_copy(out=b2rep[:], in_=ps_b2[:])
            dpre_sb = [cst.tile([P, ntile], f32, tag=f"dp{h}", name=f"dpre{h}")
                       for h in (0, 1)]
            nc.sync.dma_start(out=dpre_sb[0][:], in_=dpre0_d)
            nc.sync.dma_start(out=dpre_sb[1][:], in_=dpre1_d)

            xg = dram.tile([2 * half_rows, D], gdt, name="xg")
            ccin = [dram.tile([ws[s] * P, D], gdt, tag=f"cci{s}", name=f"cci{s}")
                    for s in range(NSLAB)]
            ccout = [dram.tile([n_cores * ws[s] * P, D], gdt, tag=f"cco{s}",
                               name=f"cco{s}", addr_space="Shared")
                     for s in range(NSLAB)]
            cc = dram.tile([pre["tot_rows"], D], gdt)

            # ---------------- precast: xg[h] = bf16(dinv * x), per half
            sgi = [0]
            for h in (0, 1):
                base_r = h * half_x
                nrows = (N - half_x) if h else half_x
                sgs = []
                r = 0
                while r + SGK * P <= nrows:
                    sgs.append((r, SGK, P))
                    r += SGK * P
                if nrows - r >= P:
                    k = (nrows - r) // P
                    sgs.append((r, k, P))
                    r += k * P
                if nrows - r > 0:
                    sgs.append((r, 1, nrows - r))
                xgv = xg[h * half_rows:(h + 1) * half_rows, :] \
                    .rearrange("(p t) f -> p t f", t=ntile)
                for (r0, k, prow) in sgs:
                    xt = pc.tile([P, SGK * P], f32, tag="xt")
                    xb = pc.tile([P, SGK * P], gdt, tag="xb")
                    src_ap = x[base_r + r0: base_r + r0 + (k - 1) * P + prow, :]
                    nc.scalar.dma_start(
                        out=xt[:prow, :k * P].rearrange("p (k f) -> p k f", f=P),
                        in_=src_ap.rearrange("(k p) f -> p k f", p=P) if prow == P
                        else src_ap.rearrange("(k p) f -> p k f", p=prow))
                    t0 = r0 // P
                    dv = dpre_sb[h][:prow, t0:t0 + k].unsqueeze(2) \
                        .broadcast_to([prow, k, P])
                    eng = nc.vector
                    sgi[0] += 1
                    eng.tensor_tensor(
                        out=xb[:prow, :k * P].rearrange("p (k f) -> p k f", f=P),
                        in0=xt[:prow, :k * P].rearrange("p (k f) -> p k f", f=P),
                        in1=dv, op=mybir.AluOpType.mult)
                    nc.sync.dma_start(
                        out=xgv[:prow, t0:t0 + k, :],
                        in_=xb[:prow, :k * P].rearrange("p (k f) -> p k f", f=P))

            # ---------------- aggregation sweep
            def agg_phase(L, in_aps, emit, group_end=None):
                cb = {h: np.r_[0, np.cumsum(kv[(L, h)])] for h in (0, 1)}
                coff = {h: 0 for h in (0, 1)}
                for gi, (a, b) in enumerate(groups):
                    Gs, Sels = {}, {}
                    for h in (0, 1):
                        n = nchg[(L, h)][gi]
                        if n == 0:
                            continue
                        tbt = tbp.tile([P, 9 * NCHMAX], i16, tag=f"tb{h}")
                        nc.sync.dma_start(
                            out=tbt[:, :9 * n],
                            in_=pk[(L, h)][:, coff[h]:coff[h] + 9 * n])
                        coff[h] += 9 * n
                        G = gg.tile([P, NCHMAX * P], gdt, tag=f"g{h}")
                        ca = max(1, n // 2)
                        for (c_lo, c_hi) in ((0, ca), (ca, n)):
                            if c_hi <= c_lo:
                                continue
                            nn = (c_hi - c_lo) * P
                            nc.gpsimd.dma_gather(
                                out_ap=G[:, c_lo * P:c_hi * P]
                                    .rearrange("p (k d) -> p k d", d=P),
                                in_ap=in_aps[h],
                                idxs_ap=tbt[:, c_lo * 8:c_hi * 8],
                                num_idxs=nn, num_idxs_reg=nn, elem_size=P,
                                single_packet=False, queue_num=next_q())
                        S = sel.tile([P, NCHMAX * P], gdt, tag=f"s{h}")
                        s3 = S[:, :n * P].rearrange("p (c j) -> p c j", j=P)
                        ds_ap = tbt[:, 8 * n:9 * n].bitcast(gdt)
                        d_b = ds_ap.unsqueeze(2).broadcast_to([P, n, P])
                        i_b = iota_sb[:, :P].unsqueeze(1).broadcast_to([P, n, P])
                        nc.vector.tensor_tensor(out=s3, in0=d_b, in1=i_b,
                                                op=mybir.AluOpType.is_equal)
                        Gs[h], Sels[h] = G, S
                    for w in range(a, b):
                        psum = psB.tile([P, P], f32, tag="pu")
                        ci = 0
                        for h in (0, 1):
                            if nchg[(L, h)][gi] == 0:
                                continue
                            k0 = int(cb[h][w] - cb[h][a])
                            for k in range(int(kv[(L, h)][w])):
                                off = (k0 + k) * P
                                gsl = Gs[h][:, off:off + P]
                                ssl = Sels[h][:, off:off + P]
                                if L == 1:
                                    nc.tensor.matmul(psum[:], lhsT=gsl, rhs=ssl,
                                                     start=(ci == 0), stop=False)
                                else:
                                    nc.tensor.matmul(psum[:], lhsT=ssl, rhs=gsl,
                                                     start=(ci == 0), stop=False)
                                ci += 1
                        emit(w, psum)
                    if group_end is not None:
                        group_end(gi, a, b)

            # ---- phase 1
            def emit_l1(w, psum_u):
                u_sb = wk.tile([P, P], gdt, tag="u")
                nc.vector.tensor_tensor(out=u_sb[:], in0=psum_u[:],
                                        in1=Dw_sb[:, w * P:(w + 1) * P],
                                        op=mybir.AluOpType.mult)
                psz1 = psA.tile([P, H], f32, tag="pz1")
                z1 = wk.tile([P, H], gdt, tag="z1")
                for j in range(JH):
                    nc.tensor.matmul(psz1[:, j * P:(j + 1) * P],
                                     lhsT=W1b[:, j * P:(j + 1) * P],
                                     rhs=u_sb[:], start=True, stop=True)
                    nc.scalar.activation(out=z1[:, j * P:(j + 1) * P],
                                         in_=psz1[:, j * P:(j + 1) * P],
                                         func=mybir.ActivationFunctionType.Relu,
                                         bias=b1_sb[:, j:j + 1])
                pst = psB.tile([P, P], f32, tag="pt")
                for j in range(JH):
                    nc.tensor.matmul(pst[:],
                                     lhsT=z1[:, j * P:(j + 1) * P],
                                     rhs=W2b[:, j * D:(j + 1) * D],
                                     start=(j == 0), stop=(j == JH - 1))
                t_sb = wk.tile([P, D], gdt, tag="t")
                nc.scalar.activation(out=t_sb[:], in_=pst[:],
                                     func=mybir.ActivationFunctionType.Copy,
                                     scale=dinvw_sb[:, w:w + 1])
                s = int(np.searchsorted(np.r_[sstart[1:], wpc], w, side="right"))
                wl = w - sstart[s]
                nc.scalar.dma_start(out=ccin[s][wl * P:(wl + 1) * P, :],
                                    in_=t_sb[:])
                if w == sstart[s] + ws[s] - 1:
                    nc.gpsimd.collective_compute(
                        "AllGather", mybir.AluOpType.bypass,
                        replica_groups=[list(range(n_cores))],
                        ins=[ccin[s][:]], outs=[ccout[s][:]])
                    nc.scalar.dma_start(
                        out=cc[outbase[s]:outbase[s] + n_cores * ws[s] * P, :],
                        in_=ccout[s][:])

            agg_phase(1, [xg[0:half_rows, :], xg[half_rows:, :]], emit_l1)

            # ---- phase 2
            zg_box = {}

            def emit_l2(w, psum_z):
                gi = w // GROUP_W
                a = groups[gi][0]
                if w == a:
                    zg_box["t"] = wk.tile([P, GROUP_W * P], f32, tag="zg",
                                          name="zg")
                nc.vector.scalar_tensor_tensor(
                    out=zg_box["t"][:, (w - a) * P:(w - a + 1) * P],
                    in0=psum_z[:], scalar=dinvw_sb[:, w:w + 1], in1=b2rep[:],
                    op0=mybir.AluOpType.mult, op1=mybir.AluOpType.add)

            def group_end_l2(gi, a, b):
                zg = zg_box["t"]
                nc.scalar.dma_start(
                    out=z_out[a * P:b * P, :].rearrange("(w p) f -> p w f", p=P),
                    in_=zg[:, :(b - a) * P].rearrange("p (w f) -> p w f", f=P))

            agg_phase(2, [cc[0:half_cc, :], cc[half_cc:, :]], emit_l2,
                      group_end=group_end_l2)

    nc.compile()
    return nc


# -------------------------------------------------------------------- kernel
def kernel(x_all, W1, b1, W2, b2, edge_index, ix=0, max_iter=10):
    x_all = np.ascontiguousarray(np.asarray(x_all, dtype=np.float32))
    W1 = np.ascontiguousarray(np.asarray(W1, dtype=np.float32))
    b1 = np.ascontiguousarray(np.asarray(b1, dtype=np.float32))
    W2 = np.ascontiguousarray(np.asarray(W2, dtype=np.float32))
    b2 = np.ascontiguousarray(np.asarray(b2, dtype=np.float32))
    edge_index = np.asarray(edge_index)

    N, D = x_all.shape
    H = W1.shape[1]
    ekey = (N, D, H, edge_index.shape[1], GROUP_W,
            int(edge_index[0, 0]), int(edge_index[1, -1]))
    if ekey in _CACHE:
        nc, pre = _CACHE[ekey]
    else:
        pre = _preprocess(edge_index, N)
        nc = _build(pre, D, H)
        _CACHE[ekey] = (nc, pre)

    JH = H // P
    b1c = b1.reshape(JH, P).T.copy()
    b2r = b2.reshape(1, D).copy()
    iota = np.ascontiguousarray(
        np.broadcast_to(np.arange(P, dtype=np.float32)[None, None, :],
                        (P, pre["NCHMAX"], P)).reshape(P, -1)) \
        .astype(ml_dtypes.bfloat16)

    in_maps = []
    for c in range(pre["n_cores"]):
        im = {"x_all": x_all, "W1": W1, "b1c": b1c, "W2": W2, "b2r": b2r,
              "iota_in": iota,
              "Dw": pre["Dw"][c], "dinvw": pre["dinvw"][c],
              "dpre0": pre["dpre"][0], "dpre1": pre["dpre"][1]}
        for L in (1, 2):
            for h in (0, 1):
                arr = pre["packed"][(L, h)][c]
                if arr.shape[1] == 0:
                    arr = np.zeros((P, 1), np.int16)
                im[f"pk{L}{h}"] = arr
        in_maps.append(im)

    res = run_bass_kernel_spmd(nc, in_maps, core_ids=list(range(pre["n_cores"])),
                               trace=TRACE)
    LAST["exec_time_ns"] = res.exec_time_ns
    LAST["mean_exec_time_ns"] = res.mean_exec_time_ns
    LAST["per_core_scope_times"] = res.per_core_scope_times
    LAST["trace_path"] = (res.instructions_and_trace or (None, None))[1]
    LAST["profile_json"] = res.profile_json

    zs = np.stack([res.results[c]["z_out"] for c in range(pre["n_cores"])])
    z = zs[pre["core_of"], pre["slot_in_core"]]
    return z.astype(np.float32)


if __name__ == "__main__":
    rng = np.random.default_rng(0)
    N, E, D, H = 4096, 40000, 128, 512
    ei = rng.integers(0, N, size=(2, E)).astype(np.int64)
    x = rng.standard_normal((N, D), dtype=np.float32)
    W1 = rng.standard_normal((D, H), dtype=np.float32) / np.sqrt(D)
    b1 = rng.standard_normal(H).astype(np.float32) * 0.1
    W2 = rng.standard_normal((H, D), dtype=np.float32) / np.sqrt(H)
    b2 = rng.standard_normal(D).astype(np.float32) * 0.1

    deg = np.bincount(ei[1], minlength=N) + 1.0
    dinv = 1.0 / np.sqrt(deg)
    asrc = np.concatenate([ei[0], np.arange(N)])
    adst = np.concatenate([ei[1], np.arange(N)])
    nrm = dinv[asrc] * dinv[adst]

    def agg(t):
        out = np.zeros_like(t)
        np.add.at(out, adst, t[asrc] * nrm[:, None])
        return out

    z1 = np.maximum(agg(x.astype(np.float64)) @ W1 + b1, 0)
    ref = agg(z1 @ W2) + b2

    got = kernel(x, W1, b1, W2, b2, ei)
    err = np.abs(got - ref)
    rel = err.max() / np.abs(ref).max()
    print(f"exec_time_ns: {LAST['exec_time_ns']}")
    print(f"max abs err {err.max():.3e}  rel(absmax) {rel:.3e}")
